# revision 11
# baseline (speedup 1.0000x reference)
"""Trainium2 Bass kernel for nn_DBLoss (YOLO-style detection loss).

Strategy (pure data parallel over batch, 8 cores x 4 images):
  * Loss = 7.5*l_box + l_obj + 0.5*l_cls.  Only the objectness term
    touches every grid cell; box/cls touch only the <=720 label-assigned
    cells per core.
  * Host (numpy) replicates the reference's target assignment on the tiny
    `labels` tensor (as in the original baseline) and builds per-core
    device inputs during sharding:
      - ch4   [128,600]  objectness logits, contiguous (one fast DMA
               instead of 70k strided 4B descriptors -- the old bottleneck)
      - posc2 [128,516]  positive-cell cls logits (class-major), selected
               correction logits, and box logits (quantity-major)
      - aux   [128,170]  per-slot box constants, correction weights,
               cls_weight
  * Device computes ALL loss math:
      - dense focal_bce(x,0) over all 76800 cells/core via ACT exp/ln
        (f0 = sigmoid^1.5 * softplus = exp(1.5*(x-l))*l, l=softplus(x))
      - the same f0 form for the 80-class focal loss at positive cells
      - a t=0 -> t=1 correction at positive (cell,channel) pairs
        (f1-f0 = exp(-1.5l)*(l-x) - exp(1.5(x-l))*l), covering both the
        objectness targets and the one-hot class targets in one pass
      - CIoU box loss on [128,12] x|y-packed tiles split across DVE+Pool,
        atan via a degree-7 odd polynomial, reciprocals via the 1-op
        approx-NR custom DVE op
      - per-partition partial sums via fused accum_out reductions
  * Host sums 8x128x4 partials (f64) and applies the loss weights and
    n_pos / mean normalizations.
"""

import sys

sys.path.insert(0, "/opt/trn_rl_repo")

import numpy as np

import concourse.bass as bass
import concourse.tile as tile
from concourse import mybir
from concourse.bass_utils import run_bass_kernel_spmd

f32 = mybir.dt.float32
AF = mybir.ActivationFunctionType
ALU = mybir.AluOpType
AX = mybir.AxisListType

# problem constants (hardcoded per harness contract)
B, NA, H, W, M, C = 32, 3, 80, 80, 20, 80
NCORES = 8
BL = B // NCORES                 # 4 images per core
NCELL = BL * NA * H * W          # 76800 cells per core
KD = NCELL // 128                # 600 dense cols
NG = 6                           # positive-slot groups: 6*128 = 768 >= 720
NSEL = 12                        # correction entries: 12*128 = 1536 >= 1440
NTOT = B * NA * H * W            # 614400 cells globally
STRIDE = np.float32(8.0)
IMG = np.float32(640.0)
EPS = np.float32(1e-7)
PI2 = np.float32(np.pi ** 2)
ANCHORS = np.array([[10.0, 13.0], [16.0, 30.0], [33.0, 23.0]], dtype=np.float32)
EMPTY_CLS = np.float32(-30.0)    # cls logit filler: f0(-30) underflows to 0

# atan(z) ~ z*(A0 + A1 z^2 + A2 z^4 + A3 z^6) on [0,1], max abs err 1.5e-4
ATAN4 = [0.99874209, -0.31793283, 0.14020638, -0.03564737]

# aux column layout
A_CXY, A_AWH, A_G1, A_G2, A_GM = 0, 12, 24, 36, 48
A_AREA, A_ATG, A_VALID, A_SELW, A_WQ = 60, 66, 72, 78, 90
AUXW = 170
# posc2 column layout: [cls(480) | sel(12) | box logits(24)]
P_SEL, P_BOX = 480, 492
PCW = 516
# partials columns
COL_OBJ, COL_CLS, COL_CORR, COL_BOX, NCOL = 0, 1, 2, 3, 4

MODE = "v3"
TRACE = False
TRACE_KW = {}
LAST_RESULT = None
_BUILD_CACHE = {}


def _split_multi_waits(nc, limit=1):
    """This container's walrus build accepts only one sync-wait per
    instruction; split Tile's stacked waits into single-wait NoOp chains."""
    n = 0
    for fn in nc.m.functions:
        for bb in fn.blocks:
            new_insts, changed = [], False
            for inst in bb.instructions:
                si = getattr(inst, "sync_info", None)
                waits = list(si.on_wait) if si is not None and si.on_wait else []
                if len(waits) > limit:
                    changed = True
                    n += 1
                    for w in waits[:-limit]:
                        nop = mybir.InstNoOp(
                            name=nc.get_next_instruction_name(),
                            engine=inst.engine,
                            sync_info=mybir.SyncInfo(on_wait=[w], on_update=[]),
                            bass_nofuse=True,
                        )
                        nc.register_instruction(nop)
                        new_insts.append(nop)
                    si.on_wait = waits[-limit:]
                new_insts.append(inst)
            if changed:
                try:
                    bb.instructions = new_insts
                except Exception:
                    bb.instructions[:] = new_insts
    return n


def _acc_stt(nc, use_accum, out_t, in0, scalar, in1, acc_col):
    """out = (in0*scalar)*in1; acc_col[:,0] = row-sum, fused or 2-op."""
    if use_accum:
        nc.vector.scalar_tensor_tensor(
            out=out_t[:], in0=in0, scalar=float(scalar), in1=in1,
            op0=ALU.mult, op1=ALU.mult, accum_out=acc_col)
    else:
        nc.vector.scalar_tensor_tensor(
            out=out_t[:], in0=in0, scalar=float(scalar), in1=in1,
            op0=ALU.mult, op1=ALU.mult)
        nc.vector.tensor_reduce(out=acc_col, in_=out_t[:], axis=AX.X,
                                op=ALU.add)


def _build_v1(use_pool=True, use_accum=True):
    nc = bass.Bass()
    ch4 = nc.declare_dram_parameter("ch4", [128, KD], f32, isOutput=False)
    posc2 = nc.declare_dram_parameter("posc2", [128, PCW], f32, isOutput=False)
    aux = nc.declare_dram_parameter("aux", [128, AUXW], f32, isOutput=False)
    outp = nc.declare_dram_parameter("out", [128, NCOL], f32, isOutput=True)

    K_V = float(np.float32(4.0) / PI2)

    with tile.TileContext(nc) as tc:
        with tc.tile_pool(name="main", bufs=1) as pool:
            PE = nc.gpsimd if use_pool else nc.vector
            # ---- input DMAs, one per HWDGE ring, all issued at t=0 ----
            x_p = pool.tile([128, PCW], f32)         # cls+sel+box logits
            nc.scalar.dma_start(out=x_p[:], in_=posc2[:])
            x_a = pool.tile([128, AUXW], f32)        # constants
            nc.sync.dma_start(out=x_a[:], in_=aux[:])
            x_o = pool.tile([128, KD], f32)          # dense obj logits
            nc.sync.dma_start(out=x_o[:], in_=ch4[:])

            partials = pool.tile([128, NCOL], f32)

            def T(name, n):
                return pool.tile([128, n], f32, name=name)

            # aux views
            cxy = x_a[:, A_CXY:A_CXY + 12]
            awh = x_a[:, A_AWH:A_AWH + 12]
            g1 = x_a[:, A_G1:A_G1 + 12]
            g2 = x_a[:, A_G2:A_G2 + 12]
            gm = x_a[:, A_GM:A_GM + 12]
            areagE = x_a[:, A_AREA:A_AREA + 6]
            atg = x_a[:, A_ATG:A_ATG + 6]
            valid = x_a[:, A_VALID:A_VALID + 6]
            selw = x_a[:, A_SELW:A_SELW + 12]
            wq80 = x_a[:, A_WQ:A_WQ + 80]
            pos4 = x_p[:, P_BOX:PCW]                  # [x0|x1|x2|x3] blocks
            xcs = x_p[:, 0:P_SEL + 12]                # cls + sel logits

            # ============ ACT: box exps first (unblocks the long chain)
            e4 = T("e4", 24)
            nc.scalar.activation(e4[:], pos4, AF.Exp)

            # ============ DVE+Pool: CIoU box loss on x|y-packed [128,12]
            e2p1 = T("e2p1", 12)
            nc.vector.tensor_scalar_add(e2p1[:], e4[:, 0:12], 1.0)
            r2 = T("r2", 12)
            nc.vector.reciprocal(out=r2[:], in_=e2p1[:])
            pxy = T("pxy", 12)                        # center coords (px|py)
            nc.vector.scalar_tensor_tensor(
                out=pxy[:], in0=r2[:], scalar=-8.0, in1=cxy,
                op0=ALU.mult, op1=ALU.add)
            pwh = T("pwh", 12)                        # box sizes (pw|ph)
            PE.tensor_tensor(out=pwh[:], in0=e4[:, 12:24], in1=awh,
                                    op=ALU.mult)
            th = T("th", 12)
            PE.tensor_scalar_mul(th[:], pwh[:], 0.5)
            p1 = T("p1", 12)
            PE.tensor_tensor(out=p1[:], in0=pxy[:], in1=th[:],
                                    op=ALU.subtract)
            p2 = T("p2", 12)
            PE.tensor_tensor(out=p2[:], in0=pxy[:], in1=th[:],
                                    op=ALU.add)
            m1 = T("m1", 12)
            nc.vector.tensor_tensor(out=m1[:], in0=p2[:], in1=g2, op=ALU.min)
            m2 = T("m2", 12)
            nc.vector.tensor_tensor(out=m2[:], in0=p1[:], in1=g1, op=ALU.max)
            iwh = T("iwh", 12)
            PE.tensor_tensor(out=iwh[:], in0=m1[:], in1=m2[:],
                                    op=ALU.subtract)
            PE.tensor_scalar_max(iwh[:], iwh[:], 0.0)
            M1 = T("M1", 12)
            nc.vector.tensor_tensor(out=M1[:], in0=p2[:], in1=g2, op=ALU.max)
            M2 = T("M2", 12)
            nc.vector.tensor_tensor(out=M2[:], in0=p1[:], in1=g1, op=ALU.min)
            cwh = T("cwh", 12)
            PE.tensor_tensor(out=cwh[:], in0=M1[:], in1=M2[:],
                                    op=ALU.subtract)
            dd = T("dd", 12)
            PE.tensor_tensor(out=dd[:], in0=pxy[:], in1=gm,
                                    op=ALU.subtract)

            inter = T("inter", 6)
            nc.vector.tensor_tensor(out=inter[:], in0=iwh[:, 0:6],
                                    in1=iwh[:, 6:12], op=ALU.mult)
            areap = T("areap", 6)
            PE.tensor_tensor(out=areap[:], in0=pwh[:, 0:6],
                                    in1=pwh[:, 6:12], op=ALU.mult)
            union = T("union", 6)
            PE.tensor_tensor(out=union[:], in0=areap[:], in1=areagE,
                                    op=ALU.add)
            nc.vector.tensor_tensor(out=union[:], in0=union[:], in1=inter[:],
                                    op=ALU.subtract)
            runi = T("runi", 6)
            nc.vector.reciprocal(out=runi[:], in_=union[:])
            iou = T("iou", 6)
            nc.vector.tensor_tensor(out=iou[:], in0=inter[:], in1=runi[:],
                                    op=ALU.mult)

            csq = T("csq", 12)
            PE.tensor_tensor(out=csq[:], in0=cwh[:], in1=cwh[:],
                                    op=ALU.mult)
            c2e = T("c2e", 6)
            PE.tensor_tensor(out=c2e[:], in0=csq[:, 0:6],
                                    in1=csq[:, 6:12], op=ALU.add)
            PE.tensor_scalar_add(c2e[:], c2e[:], float(EPS))
            rc2 = T("rc2", 6)
            nc.vector.reciprocal(out=rc2[:], in_=c2e[:])
            dsq = T("dsq", 12)
            PE.tensor_tensor(out=dsq[:], in0=dd[:], in1=dd[:],
                                    op=ALU.mult)
            rho2 = T("rho2", 6)
            PE.tensor_tensor(out=rho2[:], in0=dsq[:, 0:6],
                                    in1=dsq[:, 6:12], op=ALU.add)
            rho2c2 = T("rho2c2", 6)
            nc.vector.tensor_tensor(out=rho2c2[:], in0=rho2[:], in1=rc2[:],
                                    op=ALU.mult)

            # v = 4/pi^2 * (atan(gw/gh) - atan(pw/ph))^2 via poly atan
            phe = T("phe", 6)
            nc.vector.tensor_scalar_add(phe[:], pwh[:, 6:12], float(EPS))
            rph = T("rph", 6)
            nc.vector.reciprocal(out=rph[:], in_=phe[:])
            q = T("q", 6)
            nc.vector.tensor_tensor(out=q[:], in0=pwh[:, 0:6], in1=rph[:],
                                    op=ALU.mult)
            rq = T("rq", 6)
            nc.vector.reciprocal(out=rq[:], in_=q[:])
            z = T("z", 6)
            nc.vector.tensor_tensor(out=z[:], in0=q[:], in1=rq[:], op=ALU.min)
            z2 = T("z2", 6)
            PE.tensor_tensor(out=z2[:], in0=z[:], in1=z[:], op=ALU.mult)
            acc = T("acc", 6)
            PE.tensor_scalar(
                out=acc[:], in0=z2[:], scalar1=float(ATAN4[3]),
                scalar2=float(ATAN4[2]), op0=ALU.mult, op1=ALU.add)
            PE.tensor_tensor(out=acc[:], in0=acc[:], in1=z2[:],
                                    op=ALU.mult)
            PE.tensor_scalar_add(acc[:], acc[:], float(ATAN4[1]))
            PE.tensor_tensor(out=acc[:], in0=acc[:], in1=z2[:],
                                    op=ALU.mult)
            PE.tensor_scalar_add(acc[:], acc[:], float(ATAN4[0]))
            at0 = T("at0", 6)
            PE.tensor_tensor(out=at0[:], in0=acc[:], in1=z[:],
                                    op=ALU.mult)
            # range fix: at = at0 + (q>1)*(pi/2 - 2*at0)
            flag = T("flag", 6)
            nc.vector.tensor_scalar(
                out=flag[:], in0=q[:], scalar1=1.0, scalar2=None, op0=ALU.is_gt)
            fw = T("fw", 6)
            PE.tensor_scalar(
                out=fw[:], in0=at0[:], scalar1=-2.0,
                scalar2=float(np.pi / 2), op0=ALU.mult, op1=ALU.add)
            PE.tensor_tensor(out=fw[:], in0=fw[:], in1=flag[:],
                                    op=ALU.mult)
            at = T("at", 6)
            PE.tensor_tensor(out=at[:], in0=at0[:], in1=fw[:],
                                    op=ALU.add)
            dv = T("dv", 6)
            PE.tensor_tensor(out=dv[:], in0=atg, in1=at[:],
                                    op=ALU.subtract)
            v = T("v", 6)
            PE.tensor_tensor(out=v[:], in0=dv[:], in1=dv[:],
                                    op=ALU.mult)
            PE.tensor_scalar_mul(v[:], v[:], K_V)
            den = T("den", 6)
            nc.vector.scalar_tensor_tensor(
                out=den[:], in0=iou[:], scalar=-1.0, in1=v[:],
                op0=ALU.mult, op1=ALU.add)
            nc.vector.tensor_scalar_add(den[:], den[:], float(1.0 + float(EPS)))
            rden = T("rden", 6)
            nc.vector.reciprocal(out=rden[:], in_=den[:])
            av = T("av", 6)
            nc.vector.tensor_tensor(out=av[:], in0=v[:], in1=rden[:],
                                    op=ALU.mult)
            nc.vector.tensor_tensor(out=av[:], in0=av[:], in1=v[:],
                                    op=ALU.mult)
            li = T("li", 6)
            PE.tensor_tensor(out=li[:], in0=av[:], in1=rho2c2[:],
                                    op=ALU.add)
            nc.vector.tensor_tensor(out=li[:], in0=li[:], in1=iou[:],
                                    op=ALU.subtract)
            # per-slot loss = 1 + li; the +1*n_pos is added on host
            jb = T("jb", 6)
            _acc_stt(nc, use_accum, jb, li[:], 1.0, valid,
                     partials[:, COL_BOX:COL_BOX + 1])

            # ============ ACT/DVE: f0 = exp(1.5*(x-l))*l pipelines
            # cls+sel block [128,492]
            e_cs = T("e_cs", P_SEL + 12)
            nc.scalar.activation(e_cs[:], xcs, AF.Exp)
            l_cs = T("l_cs", P_SEL + 12)
            nc.scalar.activation(l_cs[:], e_cs[:], AF.Ln, bias=1.0)
            d_cs = T("d_cs", P_SEL + 12)
            nc.vector.tensor_tensor(out=d_cs[:], in0=xcs, in1=l_cs[:],
                                    op=ALU.subtract)
            # dense obj block [128,600]
            e_o = T("e_o", KD)
            nc.scalar.activation(e_o[:], x_o[:], AF.Exp)
            l_o = T("l_o", KD)
            nc.scalar.activation(l_o[:], e_o[:], AF.Ln, bias=1.0)
            d_o = T("d_o", KD)
            nc.vector.tensor_tensor(out=d_o[:], in0=x_o[:], in1=l_o[:],
                                    op=ALU.subtract)
            u_cs = T("u_cs", P_SEL + 12)
            nc.scalar.activation(u_cs[:], d_cs[:], AF.Exp, scale=1.5)
            u_o = T("u_o", KD)
            nc.scalar.activation(u_o[:], d_o[:], AF.Exp, scale=1.5)
            h1 = T("h1", 12)
            nc.scalar.activation(h1[:], l_cs[:, P_SEL:P_SEL + 12], AF.Exp,
                                 scale=-1.5)

            # dense obj: sum f0 = sum u*l
            jo = T("jo", KD)
            _acc_stt(nc, use_accum, jo, u_o[:], 1.0, l_o[:],
                     partials[:, COL_OBJ:COL_OBJ + 1])

            # cls + sel f0 products
            P_cs = T("P_cs", P_SEL + 12)
            nc.vector.tensor_tensor(out=P_cs[:], in0=u_cs[:], in1=l_cs[:],
                                    op=ALU.mult)
            # cls: reduce slots (class-major layout -> innermost g), then *w
            red80 = T("red80", 80)
            nc.vector.tensor_reduce(
                out=red80[:], in_=P_cs[:, 0:P_SEL].rearrange(
                    "p (c g) -> p c g", g=NG),
                axis=AX.X, op=ALU.add)
            j80 = T("j80", 80)
            _acc_stt(nc, use_accum, j80, red80[:], 1.0, wq80,
                     partials[:, COL_CLS:COL_CLS + 1])

            # corr: f1 - f0 = h1*(l-x) - P  at selected (cell,ch) pairs
            f1n = T("f1n", 12)
            PE.tensor_tensor(out=f1n[:], in0=h1[:],
                                    in1=d_cs[:, P_SEL:P_SEL + 12],
                                    op=ALU.mult)
            ncor = T("ncor", 12)
            PE.tensor_tensor(out=ncor[:], in0=f1n[:],
                                    in1=P_cs[:, P_SEL:P_SEL + 12],
                                    op=ALU.add)
            jc = T("jc", 12)
            _acc_stt(nc, use_accum, jc, ncor[:], -1.0, selw,
                     partials[:, COL_CORR:COL_CORR + 1])

            # ---- store per-partition partials; host reduces across cores
            nc.sync.dma_start(out=outp[:], in_=partials[:])

    _split_multi_waits(nc)
    return nc




def _build_v2():
    """All-DVE box chain with fused/packed ops; Pool runs only the atan
    polynomial and corr product branches; all bulk DMAs on the ACT ring
    (the sync-ring DMA queue is packet-rate-limited ~25M pkt/s)."""
    nc = bass.Bass()
    ch4 = nc.declare_dram_parameter("ch4", [128, KD], f32, isOutput=False)
    posc2 = nc.declare_dram_parameter("posc2", [128, PCW], f32, isOutput=False)
    aux = nc.declare_dram_parameter("aux", [128, AUXW], f32, isOutput=False)
    outp = nc.declare_dram_parameter("out", [128, NCOL], f32, isOutput=True)

    K_V = float(np.float32(4.0) / PI2)

    with tile.TileContext(nc) as tc:
        with tc.tile_pool(name="main", bufs=1) as pool:
            x_p = pool.tile([128, PCW], f32)
            nc.scalar.dma_start(out=x_p[:], in_=posc2[:])
            x_a = pool.tile([128, AUXW], f32)
            nc.scalar.dma_start(out=x_a[:], in_=aux[:])
            x_o = pool.tile([128, KD], f32)
            nc.scalar.dma_start(out=x_o[:], in_=ch4[:])

            partials = pool.tile([128, NCOL], f32)

            def T(name, n):
                return pool.tile([128, n], f32, name=name)

            cxy = x_a[:, A_CXY:A_CXY + 12]
            awh = x_a[:, A_AWH:A_AWH + 12]
            g1 = x_a[:, A_G1:A_G1 + 12]
            g2 = x_a[:, A_G2:A_G2 + 12]
            gm = x_a[:, A_GM:A_GM + 12]
            areagE = x_a[:, A_AREA:A_AREA + 6]
            atg = x_a[:, A_ATG:A_ATG + 6]
            valid = x_a[:, A_VALID:A_VALID + 6]
            selw = x_a[:, A_SELW:A_SELW + 12]
            wq80 = x_a[:, A_WQ:A_WQ + 80]
            pos4 = x_p[:, P_BOX:PCW]
            xcs = x_p[:, 0:P_SEL + 12]

            # ============ ACT: box exps first
            e4 = T("e4", 24)
            nc.scalar.activation(e4[:], pos4, AF.Exp)

            # ============ DVE box chain (x|y packed [128,12])
            e2p1 = T("e2p1", 12)
            nc.vector.tensor_scalar_add(e2p1[:], e4[:, 0:12], 1.0)
            r2 = T("r2", 12)
            nc.vector.reciprocal(out=r2[:], in_=e2p1[:])
            pxy = T("pxy", 12)
            nc.vector.scalar_tensor_tensor(
                out=pxy[:], in0=r2[:], scalar=-8.0, in1=cxy,
                op0=ALU.mult, op1=ALU.add)
            pwh = T("pwh", 12)
            nc.vector.tensor_tensor(out=pwh[:], in0=e4[:, 12:24], in1=awh,
                                    op=ALU.mult)
            th = T("th", 12)
            nc.vector.tensor_scalar_mul(th[:], pwh[:], 0.5)
            p1 = T("p1", 12)
            nc.vector.tensor_tensor(out=p1[:], in0=pxy[:], in1=th[:],
                                    op=ALU.subtract)
            p2 = T("p2", 12)
            nc.vector.tensor_tensor(out=p2[:], in0=pxy[:], in1=th[:],
                                    op=ALU.add)
            # rwh = 1/pwh for both q and qi (ph,pw >= 0.03 always; no EPS)
            rwh = T("rwh", 12)
            nc.vector.reciprocal(out=rwh[:], in_=pwh[:])
            # packed [min|max] pairs -> one subtract gives [iw_raw | cw]
            mM1 = T("mM1", 24)
            nc.vector.tensor_tensor(out=mM1[:, 0:12], in0=p2[:], in1=g2,
                                    op=ALU.min)
            nc.vector.tensor_tensor(out=mM1[:, 12:24], in0=p2[:], in1=g2,
                                    op=ALU.max)
            mM2 = T("mM2", 24)
            nc.vector.tensor_tensor(out=mM2[:, 0:12], in0=p1[:], in1=g1,
                                    op=ALU.max)
            nc.vector.tensor_tensor(out=mM2[:, 12:24], in0=p1[:], in1=g1,
                                    op=ALU.min)
            dif = T("dif", 24)
            nc.vector.tensor_tensor(out=dif[:], in0=mM1[:], in1=mM2[:],
                                    op=ALU.subtract)
            iwh = T("iwh", 12)
            nc.vector.tensor_scalar_max(iwh[:], dif[:, 0:12], 0.0)
            # Pool branch A: q/z/atan polynomial (independent after rwh/pwh)
            q6 = T("q6", 12)                     # [q | qi]
            nc.gpsimd.tensor_tensor(out=q6[:, 0:6], in0=pwh[:, 0:6],
                                    in1=rwh[:, 6:12], op=ALU.mult)
            nc.gpsimd.tensor_tensor(out=q6[:, 6:12], in0=pwh[:, 6:12],
                                    in1=rwh[:, 0:6], op=ALU.mult)
            z = T("z", 6)
            nc.vector.tensor_tensor(out=z[:], in0=q6[:, 0:6], in1=q6[:, 6:12],
                                    op=ALU.min)
            z2 = T("z2", 6)
            nc.gpsimd.tensor_tensor(out=z2[:], in0=z[:], in1=z[:],
                                    op=ALU.mult)
            acc = T("acc", 6)
            nc.gpsimd.tensor_scalar(
                out=acc[:], in0=z2[:], scalar1=float(ATAN4[3]),
                scalar2=float(ATAN4[2]), op0=ALU.mult, op1=ALU.add)
            nc.gpsimd.tensor_tensor(out=acc[:], in0=acc[:], in1=z2[:],
                                    op=ALU.mult)
            nc.gpsimd.tensor_scalar_add(acc[:], acc[:], float(ATAN4[1]))
            nc.gpsimd.tensor_tensor(out=acc[:], in0=acc[:], in1=z2[:],
                                    op=ALU.mult)
            nc.gpsimd.tensor_scalar_add(acc[:], acc[:], float(ATAN4[0]))
            at0 = T("at0", 6)
            nc.gpsimd.tensor_tensor(out=at0[:], in0=acc[:], in1=z[:],
                                    op=ALU.mult)
            flag = T("flag", 6)
            nc.gpsimd.tensor_scalar(
                out=flag[:], in0=q6[:, 0:6], scalar1=1.0, scalar2=None,
                op0=ALU.is_gt)
            fw = T("fw", 6)
            nc.gpsimd.tensor_scalar(
                out=fw[:], in0=at0[:], scalar1=-2.0,
                scalar2=float(np.pi / 2), op0=ALU.mult, op1=ALU.add)
            nc.gpsimd.tensor_tensor(out=fw[:], in0=fw[:], in1=flag[:],
                                    op=ALU.mult)
            at = T("at", 6)
            nc.gpsimd.tensor_tensor(out=at[:], in0=at0[:], in1=fw[:],
                                    op=ALU.add)
            dv = T("dv", 6)
            nc.gpsimd.tensor_tensor(out=dv[:], in0=atg, in1=at[:],
                                    op=ALU.subtract)
            v = T("v", 6)
            nc.gpsimd.tensor_tensor(out=v[:], in0=dv[:], in1=dv[:],
                                    op=ALU.mult)
            nc.gpsimd.tensor_scalar_mul(v[:], v[:], K_V)
            # DVE main: inter/union/c2/rho2
            inter = T("inter", 6)
            nc.vector.tensor_tensor(out=inter[:], in0=iwh[:, 0:6],
                                    in1=iwh[:, 6:12], op=ALU.mult)
            areap = T("areap", 6)
            nc.gpsimd.tensor_tensor(out=areap[:], in0=rcin[:, 12:18],
                                    in1=rcin[:, 18:24], op=ALU.mult)
            ucb = T("ucb", 12)                   # [union | c2]
            nc.vector.tensor_tensor(out=ucb[:, 0:6], in0=areap[:],
                                    in1=areagE, op=ALU.add)
            nc.vector.tensor_tensor(out=ucb[:, 0:6], in0=ucb[:, 0:6],
                                    in1=inter[:], op=ALU.subtract)
            csq = T("csq", 12)
            nc.vector.tensor_tensor(out=csq[:], in0=dif[:, 12:24],
                                    in1=dif[:, 12:24], op=ALU.mult)
            nc.vector.tensor_tensor(out=ucb[:, 6:12], in0=csq[:, 0:6],
                                    in1=csq[:, 6:12], op=ALU.add)
            rb = T("rb", 12)                     # [1/union | 1/c2]
            nc.vector.reciprocal(out=rb[:], in_=ucb[:])
            iou = T("iou", 6)
            nc.vector.tensor_tensor(out=iou[:], in0=inter[:], in1=rb[:, 0:6],
                                    op=ALU.mult)
            dd = T("dd", 12)
            nc.vector.tensor_tensor(out=dd[:], in0=pxy[:], in1=gm,
                                    op=ALU.subtract)
            dsq = T("dsq", 12)
            nc.vector.tensor_tensor(out=dsq[:], in0=dd[:], in1=dd[:],
                                    op=ALU.mult)
            rho2 = T("rho2", 6)
            nc.vector.tensor_tensor(out=rho2[:], in0=dsq[:, 0:6],
                                    in1=dsq[:, 6:12], op=ALU.add)
            rho2c2 = T("rho2c2", 6)
            nc.vector.tensor_tensor(out=rho2c2[:], in0=rho2[:],
                                    in1=rb[:, 6:12], op=ALU.mult)
            den = T("den", 6)
            nc.vector.scalar_tensor_tensor(
                out=den[:], in0=iou[:], scalar=-1.0, in1=v[:],
                op0=ALU.mult, op1=ALU.add)
            nc.vector.tensor_scalar_add(den[:], den[:], float(1.0 + float(EPS)))
            rden = T("rden", 6)
            nc.vector.reciprocal(out=rden[:], in_=den[:])
            av = T("av", 6)
            nc.vector.tensor_tensor(out=av[:], in0=v[:], in1=rden[:],
                                    op=ALU.mult)
            nc.vector.tensor_tensor(out=av[:], in0=av[:], in1=v[:],
                                    op=ALU.mult)
            li = T("li", 6)
            nc.vector.tensor_tensor(out=li[:], in0=av[:], in1=rho2c2[:],
                                    op=ALU.add)
            nc.vector.tensor_tensor(out=li[:], in0=li[:], in1=iou[:],
                                    op=ALU.subtract)
            jb = T("jb", 6)
            nc.vector.scalar_tensor_tensor(
                out=jb[:], in0=li[:], scalar=1.0, in1=valid,
                op0=ALU.mult, op1=ALU.mult)
            nc.vector.tensor_reduce(
                out=partials[:, COL_BOX:COL_BOX + 1], in_=jb[:], axis=AX.X,
                op=ALU.add)

            # ============ f0 pipelines (ACT exp/ln + DVE)
            e_cs = T("e_cs", P_SEL + 12)
            nc.scalar.activation(e_cs[:], xcs, AF.Exp)
            l_cs = T("l_cs", P_SEL + 12)
            nc.scalar.activation(l_cs[:], e_cs[:], AF.Ln, bias=1.0)
            d_cs = T("d_cs", P_SEL + 12)
            nc.vector.tensor_tensor(out=d_cs[:], in0=xcs, in1=l_cs[:],
                                    op=ALU.subtract)
            e_o = T("e_o", KD)
            nc.scalar.activation(e_o[:], x_o[:], AF.Exp)
            l_o = T("l_o", KD)
            nc.scalar.activation(l_o[:], e_o[:], AF.Ln, bias=1.0)
            d_o = T("d_o", KD)
            nc.vector.tensor_tensor(out=d_o[:], in0=x_o[:], in1=l_o[:],
                                    op=ALU.subtract)
            u_cs = T("u_cs", P_SEL + 12)
            nc.scalar.activation(u_cs[:], d_cs[:], AF.Exp, scale=1.5)
            u_o = T("u_o", KD)
            nc.scalar.activation(u_o[:], d_o[:], AF.Exp, scale=1.5)
            h1 = T("h1", 12)
            nc.scalar.activation(h1[:], l_cs[:, P_SEL:P_SEL + 12], AF.Exp,
                                 scale=-1.5)

            jo = T("jo", KD)
            nc.vector.tensor_tensor(out=jo[:], in0=u_o[:], in1=l_o[:],
                                    op=ALU.mult)
            nc.vector.tensor_reduce(
                out=partials[:, COL_OBJ:COL_OBJ + 1], in_=jo[:], axis=AX.X,
                op=ALU.add)

            P_cs = T("P_cs", P_SEL + 12)
            nc.vector.tensor_tensor(out=P_cs[:], in0=u_cs[:], in1=l_cs[:],
                                    op=ALU.mult)
            red80 = T("red80", 80)
            nc.vector.tensor_reduce(
                out=red80[:], in_=P_cs[:, 0:P_SEL].rearrange(
                    "p (c g) -> p c g", g=NG),
                axis=AX.X, op=ALU.add)
            j80 = T("j80", 80)
            nc.vector.tensor_tensor(out=j80[:], in0=red80[:], in1=wq80,
                                    op=ALU.mult)
            nc.vector.tensor_reduce(
                out=partials[:, COL_CLS:COL_CLS + 1], in_=j80[:], axis=AX.X,
                op=ALU.add)

            # corr on Pool (2 ops), final weighted reduce on DVE
            f1n = T("f1n", 12)
            nc.gpsimd.tensor_tensor(out=f1n[:], in0=h1[:],
                                    in1=d_cs[:, P_SEL:P_SEL + 12],
                                    op=ALU.mult)
            ncor = T("ncor", 12)
            nc.gpsimd.tensor_tensor(out=ncor[:], in0=f1n[:],
                                    in1=P_cs[:, P_SEL:P_SEL + 12],
                                    op=ALU.add)
            jc = T("jc", 12)
            nc.vector.scalar_tensor_tensor(
                out=jc[:], in0=ncor[:], scalar=-1.0, in1=selw,
                op0=ALU.mult, op1=ALU.mult)
            nc.vector.tensor_reduce(
                out=partials[:, COL_CORR:COL_CORR + 1], in_=jc[:], axis=AX.X,
                op=ALU.add)

            nc.sync.dma_start(out=outp[:], in_=partials[:])

    _split_multi_waits(nc)
    return nc




# V3 aux layout (f32)
B_POS4, B_CXY, B_AWH, B_G1, B_G2, B_GM = 0, 24, 36, 48, 60, 72
B_AREA, B_ATGX, B_VALID, B_SELW, B_WQ = 84, 90, 96, 102, 114
AUX3 = 194
# big (bf16): [cls(480) | sel(12) | ch4(600)]
BIGW = 1092
bf16 = mybir.dt.bfloat16
# atan deg-5 odd poly on [0,1], max err 1.0e-3
ATAN5 = [0.9931425, -0.28070902, 0.07320315]


def _build_v3():
    """bf16 data path, merged exp/ln/u mega-ops, host-selected atan branch
    (no flag ops), fused squares, aux-first DMA so the box chain starts
    as early as possible."""
    nc = bass.Bass()
    aux = nc.declare_dram_parameter("aux", [128, AUX3], f32, isOutput=False)
    big = nc.declare_dram_parameter("big", [128, BIGW], bf16, isOutput=False)
    outp = nc.declare_dram_parameter("out", [128, NCOL], f32, isOutput=True)

    K_V = float(np.float32(4.0) / PI2)

    with tile.TileContext(nc) as tc:
        with tc.tile_pool(name="main", bufs=1) as pool:
            x_a = pool.tile([128, AUX3], f32)
            nc.scalar.dma_start(out=x_a[:], in_=aux[:])
            x_b = pool.tile([128, BIGW], bf16)
            nc.scalar.dma_start(out=x_b[:], in_=big[:])

            partials = pool.tile([128, NCOL], f32)

            def T(name, n, dt=f32):
                return pool.tile([128, n], dt, name=name)

            pos4 = x_a[:, B_POS4:B_POS4 + 24]
            cxy = x_a[:, B_CXY:B_CXY + 12]
            awh = x_a[:, B_AWH:B_AWH + 12]
            g1 = x_a[:, B_G1:B_G1 + 12]
            g2 = x_a[:, B_G2:B_G2 + 12]
            gm = x_a[:, B_GM:B_GM + 12]
            areagE = x_a[:, B_AREA:B_AREA + 6]
            atgx = x_a[:, B_ATGX:B_ATGX + 6]
            valid = x_a[:, B_VALID:B_VALID + 6]
            selw = x_a[:, B_SELW:B_SELW + 12]
            wq80 = x_a[:, B_WQ:B_WQ + 80]

            # ---- ACT: box exps + (e4+1) for the sigmoid reciprocals
            e4 = T("e4", 24)
            nc.scalar.activation(e4[:], pos4, AF.Exp)
            rcin = T("rcin", 24)               # [e4+1 | pwh]
            nc.scalar.activation(rcin[:, 0:12], e4[:, 0:12], AF.Identity,
                                 bias=1.0)

            # ---- DVE box chain (rcin[12:24]=pwh written by DVE, then one
            # reciprocal covers both 1/(1+e) and 1/pwh)
            pwh = rcin[:, 12:24]
            nc.vector.tensor_tensor(out=pwh, in0=e4[:, 12:24], in1=awh,
                                    op=ALU.mult)
            rc = T("rc", 24)                   # [r2 | rwh]
            nc.vector.reciprocal(out=rc[:], in_=rcin[:])
            pxy = T("pxy", 12)
            nc.vector.scalar_tensor_tensor(
                out=pxy[:], in0=rc[:, 0:12], scalar=-8.0, in1=cxy,
                op0=ALU.mult, op1=ALU.add)
            p1 = T("p1", 12)
            nc.vector.scalar_tensor_tensor(
                out=p1[:], in0=pwh, scalar=-0.5, in1=pxy[:],
                op0=ALU.mult, op1=ALU.add)
            p2 = T("p2", 12)
            nc.vector.scalar_tensor_tensor(
                out=p2[:], in0=pwh, scalar=0.5, in1=pxy[:],
                op0=ALU.mult, op1=ALU.add)
            mM1 = T("mM1", 24)
            nc.vector.tensor_tensor(out=mM1[:, 0:12], in0=p2[:], in1=g2,
                                    op=ALU.min)
            nc.vector.tensor_tensor(out=mM1[:, 12:24], in0=p2[:], in1=g2,
                                    op=ALU.max)
            mM2 = T("mM2", 24)
            nc.vector.tensor_tensor(out=mM2[:, 0:12], in0=p1[:], in1=g1,
                                    op=ALU.max)
            nc.vector.tensor_tensor(out=mM2[:, 12:24], in0=p1[:], in1=g1,
                                    op=ALU.min)
            # sqin = [iw_raw | cw | dd]; one 36-wide square covers all
            sqin = T("sqin", 36)
            nc.vector.tensor_tensor(out=sqin[:, 0:24], in0=mM1[:],
                                    in1=mM2[:], op=ALU.subtract)
            nc.gpsimd.tensor_tensor(out=sqin[:, 24:36], in0=pxy[:], in1=gm,
                                    op=ALU.subtract)
            sqv = T("sqv", 36)
            nc.vector.tensor_tensor(out=sqv[:, 12:24], in0=sqin[:, 12:24],
                                    in1=sqin[:, 12:24], op=ALU.mult)
            nc.gpsimd.tensor_tensor(out=sqv[:, 24:36], in0=sqin[:, 24:36],
                                    in1=sqin[:, 24:36], op=ALU.mult)
            iwh = T("iwh", 12)
            nc.vector.tensor_scalar_max(iwh[:], sqin[:, 0:12], 0.0)
            inter = T("inter", 6)
            nc.vector.tensor_tensor(out=inter[:], in0=iwh[:, 0:6],
                                    in1=iwh[:, 6:12], op=ALU.mult)
            areap = T("areap", 6)
            nc.gpsimd.tensor_tensor(out=areap[:], in0=rcin[:, 12:18],
                                    in1=rcin[:, 18:24], op=ALU.mult)
            ucb = T("ucb", 12)
            nc.gpsimd.tensor_tensor(out=ucb[:, 0:6], in0=areap[:],
                                    in1=areagE, op=ALU.add)
            nc.vector.tensor_tensor(out=ucb[:, 0:6], in0=ucb[:, 0:6],
                                    in1=inter[:], op=ALU.subtract)
            nc.vector.tensor_tensor(out=ucb[:, 6:12], in0=sqv[:, 12:18],
                                    in1=sqv[:, 18:24], op=ALU.add)
            rb = T("rb", 12)
            nc.vector.reciprocal(out=rb[:], in_=ucb[:])
            iou = T("iou", 6)
            nc.vector.tensor_tensor(out=iou[:], in0=inter[:], in1=rb[:, 0:6],
                                    op=ALU.mult)
            rho2 = T("rho2", 6)
            nc.gpsimd.tensor_tensor(out=rho2[:], in0=sqv[:, 24:30],
                                    in1=sqv[:, 30:36], op=ALU.add)
            rho2c2 = T("rho2c2", 6)
            nc.vector.tensor_tensor(out=rho2c2[:], in0=rho2[:],
                                    in1=rb[:, 6:12], op=ALU.mult)
            # v branch: z = min(q, 1/q); q = pw/ph (pw,ph >= 0.03, no EPS)
            q6 = T("q6", 12)
            nc.gpsimd.tensor_tensor(out=q6[:, 0:6], in0=rcin[:, 12:18],
                                    in1=rc[:, 18:24], op=ALU.mult)
            nc.gpsimd.tensor_tensor(out=q6[:, 6:12], in0=rcin[:, 18:24],
                                    in1=rc[:, 12:18], op=ALU.mult)
            z = T("z", 6)
            nc.vector.tensor_tensor(out=z[:], in0=q6[:, 0:6], in1=q6[:, 6:12],
                                    op=ALU.min)
            # Pool: z2 + odd poly -> at0 = atan(z)
            z2 = T("z2", 6)
            nc.gpsimd.tensor_tensor(out=z2[:], in0=z[:], in1=z[:],
                                    op=ALU.mult)
            acc = T("acc", 6)
            nc.gpsimd.tensor_scalar(
                out=acc[:], in0=z2[:], scalar1=float(ATAN5[2]),
                scalar2=float(ATAN5[1]), op0=ALU.mult, op1=ALU.add)
            nc.gpsimd.tensor_tensor(out=acc[:], in0=acc[:], in1=z2[:],
                                    op=ALU.mult)
            nc.gpsimd.tensor_scalar_add(acc[:], acc[:], float(ATAN5[0]))
            at0 = T("at0", 6)
            nc.gpsimd.tensor_tensor(out=at0[:], in0=acc[:], in1=z[:],
                                    op=ALU.mult)
            # host pre-selected target angle (atg or pi/2-atg): sign of the
            # difference cancels in the square, so no range-fix ops needed
            dvx = T("dvx", 6)
            nc.vector.tensor_tensor(out=dvx[:], in0=at0[:], in1=atgx,
                                    op=ALU.subtract)
            vsq = T("vsq", 6)
            nc.vector.tensor_tensor(out=vsq[:], in0=dvx[:], in1=dvx[:],
                                    op=ALU.mult)
            vp1 = T("vp1", 6)
            nc.vector.tensor_scalar(
                out=vp1[:], in0=vsq[:], scalar1=K_V,
                scalar2=float(1.0 + float(EPS)), op0=ALU.mult, op1=ALU.add)
            v2k = T("v2k", 6)
            nc.vector.tensor_tensor(out=v2k[:], in0=vsq[:], in1=vsq[:],
                                    op=ALU.mult)
            den = T("den", 6)
            nc.vector.scalar_tensor_tensor(
                out=den[:], in0=iou[:], scalar=-1.0, in1=vp1[:],
                op0=ALU.mult, op1=ALU.add)
            rden = T("rden", 6)
            nc.vector.reciprocal(out=rden[:], in_=den[:])
            av = T("av", 6)
            nc.vector.scalar_tensor_tensor(
                out=av[:], in0=v2k[:], scalar=float(K_V * K_V), in1=rden[:],
                op0=ALU.mult, op1=ALU.mult)
            li = T("li", 6)
            nc.vector.tensor_tensor(out=li[:], in0=av[:], in1=rho2c2[:],
                                    op=ALU.add)
            nc.vector.tensor_tensor(out=li[:], in0=li[:], in1=iou[:],
                                    op=ALU.subtract)
            jb = T("jb", 6)
            nc.vector.scalar_tensor_tensor(
                out=jb[:], in0=li[:], scalar=1.0, in1=valid,
                op0=ALU.mult, op1=ALU.mult)
            nc.vector.tensor_reduce(
                out=partials[:, COL_BOX:COL_BOX + 1], in_=jb[:], axis=AX.X,
                op=ALU.add)

            # ---- merged f0 pipeline over [cls|sel|ch4] (bf16)
            e_all = T("e_all", BIGW, bf16)
            nc.scalar.activation(e_all[:], x_b[:], AF.Exp)
            l_all = T("l_all", BIGW, bf16)
            nc.scalar.activation(l_all[:], e_all[:], AF.Ln, bias=1.0)
            d_all = T("d_all", BIGW, bf16)
            nc.vector.tensor_tensor(out=d_all[:], in0=x_b[:], in1=l_all[:],
                                    op=ALU.subtract)
            u_all = T("u_all", BIGW, bf16)
            nc.scalar.activation(u_all[:], d_all[:], AF.Exp, scale=1.5)
            h1 = T("h1", 12, bf16)
            nc.scalar.activation(h1[:], l_all[:, P_SEL:P_SEL + 12], AF.Exp,
                                 scale=-1.5)
            P_all = T("P_all", BIGW, bf16)
            nc.vector.tensor_tensor(out=P_all[:], in0=u_all[:], in1=l_all[:],
                                    op=ALU.mult)
            # dense obj = sum over ch4 block
            nc.vector.tensor_reduce(
                out=partials[:, COL_OBJ:COL_OBJ + 1],
                in_=P_all[:, P_SEL + 12:BIGW], axis=AX.X, op=ALU.add)
            # cls: reduce slots (class-major, g innermost), then * weights
            red80 = T("red80", 80)
            nc.vector.tensor_reduce(
                out=red80[:], in_=P_all[:, 0:P_SEL].rearrange(
                    "p (c g) -> p c g", g=NG),
                axis=AX.X, op=ALU.add)
            j80 = T("j80", 80)
            nc.vector.tensor_tensor(out=j80[:], in0=red80[:], in1=wq80,
                                    op=ALU.mult)
            nc.vector.tensor_reduce(
                out=partials[:, COL_CLS:COL_CLS + 1], in_=j80[:], axis=AX.X,
                op=ALU.add)
            # corr: -(h1*d + P) * selw summed
            f1n = T("f1n", 12, bf16)
            nc.vector.tensor_tensor(out=f1n[:], in0=h1[:],
                                    in1=d_all[:, P_SEL:P_SEL + 12],
                                    op=ALU.mult)
            ncor = T("ncor", 12, bf16)
            nc.vector.tensor_tensor(out=ncor[:], in0=f1n[:],
                                    in1=P_all[:, P_SEL:P_SEL + 12],
                                    op=ALU.add)
            ncm = T("ncm", 12)
            nc.vector.tensor_scalar_mul(ncm[:], ncor[:], -1.0)
            jc = T("jc", 12)
            nc.vector.tensor_tensor(out=jc[:], in0=ncm[:], in1=selw,
                                    op=ALU.mult)
            nc.vector.tensor_reduce(
                out=partials[:, COL_CORR:COL_CORR + 1], in_=jc[:], axis=AX.X,
                op=ALU.add)

            nc.sync.dma_start(out=outp[:], in_=partials[:])

    _split_multi_waits(nc)
    return nc


def _build(mode):
    if mode == "v1nopool":
        return _build_v1(use_pool=False, use_accum=False)
    if mode == "v1min":
        return _build_v1(use_pool=False, use_accum=False)
    if mode == "v1accum":
        return _build_v1(use_accum=True)
    if mode == "v1":
        return _build_v1(use_accum=False)
    if mode == "v2":
        return _build_v2()
    # default: v3
    return _build_v3()


def _host_prepare(p_raw, labels, label_mask, cls_weight):
    """Replicate reference.assign_targets on host; build per-core device
    inputs.  Returns (ch4, posc2, aux, n_targets, n_pos)."""
    labels = np.asarray(labels, dtype=np.float32)
    mask = np.asarray(label_mask).astype(bool)
    cw = np.asarray(cls_weight, dtype=np.float32)

    gcls = labels[..., 0].astype(np.int32)
    gx = labels[..., 1] * IMG
    gy = labels[..., 2] * IMG
    gw = labels[..., 3] * IMG
    gh = labels[..., 4] * IMG
    gi = np.clip(gx / STRIDE, np.float32(0.0),
                 np.float32(W - 0.001)).astype(np.int32)
    gj = np.clip(gy / STRIDE, np.float32(0.0),
                 np.float32(H - 0.001)).astype(np.int32)
    gtw, gth = gw / STRIDE, gh / STRIDE
    ag = ANCHORS / STRIDE
    inter = (np.minimum(gtw[..., None], ag[:, 0])
             * np.minimum(gth[..., None], ag[:, 1]))
    union = (gtw[..., None] * gth[..., None] + ag[:, 0] * ag[:, 1]
             - inter + np.float32(1e-9))
    best_a = np.argmax(inter / union, axis=-1).astype(np.int32)

    offs = [(di, dj) for di in (-1, 0, 1) for dj in (-1, 0, 1)]
    # ordered scatter: tbox last-write-wins, tcls accumulates the class set
    targets = {}  # (b, a, j, i) -> [set(cls), (bx, by, bw, bh)]
    for b in range(B):
        for m in range(M):
            if not mask[b, m]:
                continue
            a = int(best_a[b, m])
            c = int(gcls[b, m])
            box = (gx[b, m], gy[b, m], gw[b, m], gh[b, m])
            for di, dj in offs:
                i = min(max(int(gi[b, m]) + di, 0), W - 1)
                j = min(max(int(gj[b, m]) + dj, 0), H - 1)
                e = targets.setdefault((b, a, j, i), [set(), None])
                e[0].add(c)
                e[1] = box
    n_targets = len(targets)
    n_pos = max(n_targets, 1)

    ch4 = np.ascontiguousarray(
        np.asarray(p_raw, dtype=np.float32)[..., 4]
    ).reshape(NCORES, 128, KD)

    pr = np.asarray(p_raw, dtype=np.float32).reshape(NCORES, BL, NA, H, W,
                                                     5 + C)
    posc = np.full((NCORES, 128, C, NG), EMPTY_CLS, dtype=np.float32)
    sel = np.zeros((NCORES, 128, NSEL), dtype=np.float32)
    box4 = np.zeros((NCORES, 128, 4, NG), dtype=np.float32)
    aux = np.zeros((NCORES, 128, AUXW), dtype=np.float32)
    aux[:, :, A_AWH:A_AWH + 12] = 1.0        # empty slots: pw=ph=1 (no /0)
    aux[:, :, A_AREA:A_AREA + 6] = float(EPS)
    aux[:, :, A_WQ:A_WQ + 80] = cw

    w_obj = 0.25 / float(NTOT)
    w_cls = 0.125 / (float(n_pos) * C)

    slot_ctr = [0] * NCORES
    sel_ctr = [0] * NCORES
    for (b, a, j, i), (clsset, box) in targets.items():
        core = b // BL
        s = slot_ctr[core]
        slot_ctr[core] += 1
        assert s < 128 * NG, "positive-slot capacity exceeded"
        p_, g_ = s % 128, s // 128
        bloc = b - core * BL
        row = pr[core, bloc, a, j, i]
        box4[core, p_, :, g_] = row[0:4]
        posc[core, p_, :, g_] = row[5:]
        bx, by, bw, bh = box
        gx1 = bx - bw * np.float32(0.5)
        gx2 = bx + bw * np.float32(0.5)
        gy1 = by - bh * np.float32(0.5)
        gy2 = by + bh * np.float32(0.5)
        areag = (max(gx2 - gx1, np.float32(0.0))
                 * max(gy2 - gy1, np.float32(0.0)))
        au = aux[core, p_]
        au[A_CXY + g_] = 8.0 * i + 8.0
        au[A_CXY + 6 + g_] = 8.0 * j + 8.0
        au[A_AWH + g_] = ANCHORS[a, 0]
        au[A_AWH + 6 + g_] = ANCHORS[a, 1]
        au[A_G1 + g_] = gx1
        au[A_G1 + 6 + g_] = gy1
        au[A_G2 + g_] = gx2
        au[A_G2 + 6 + g_] = gy2
        au[A_GM + g_] = bx
        au[A_GM + 6 + g_] = by
        au[A_AREA + g_] = areag + EPS
        au[A_ATG + g_] = np.arctan(bw / (bh + EPS))
        au[A_VALID + g_] = 1.0
        # correction entries: objectness (t=1) + each target class (t=1)
        t = sel_ctr[core]
        sel_ctr[core] += 1 + len(clsset)
        assert sel_ctr[core] <= 128 * NSEL, "correction capacity exceeded"
        sel[core, t % 128, t // 128] = row[4]
        aux[core, t % 128, A_SELW + t // 128] = w_obj
        for c in clsset:
            t += 1
            sel[core, t % 128, t // 128] = row[5 + c]
            aux[core, t % 128, A_SELW + t // 128] = w_cls * cw[c]

    posc2 = np.concatenate(
        [posc.reshape(NCORES, 128, C * NG), sel,
         box4.reshape(NCORES, 128, 4 * NG)], axis=2)
    return ch4, np.ascontiguousarray(posc2), aux, n_targets, n_pos




def _host_prepare_v3(p_raw, labels, label_mask, cls_weight):
    import ml_dtypes
    ch4, posc2, aux, n_targets, n_pos = _host_prepare(
        p_raw, labels, label_mask, cls_weight)
    aux3 = np.zeros((NCORES, 128, AUX3), dtype=np.float32)
    aux3[:, :, B_POS4:B_POS4 + 24] = posc2[:, :, P_BOX:PCW]
    aux3[:, :, B_CXY:B_CXY + 12] = aux[:, :, A_CXY:A_CXY + 12]
    aux3[:, :, B_AWH:B_AWH + 12] = aux[:, :, A_AWH:A_AWH + 12]
    aux3[:, :, B_G1:B_G1 + 12] = aux[:, :, A_G1:A_G1 + 12]
    aux3[:, :, B_G2:B_G2 + 12] = aux[:, :, A_G2:A_G2 + 12]
    aux3[:, :, B_GM:B_GM + 12] = aux[:, :, A_GM:A_GM + 12]
    aux3[:, :, B_AREA:B_AREA + 6] = aux[:, :, A_AREA:A_AREA + 6]
    aux3[:, :, B_VALID:B_VALID + 6] = aux[:, :, A_VALID:A_VALID + 6]
    aux3[:, :, B_SELW:B_SELW + 12] = aux[:, :, A_SELW:A_SELW + 12]
    aux3[:, :, B_WQ:B_WQ + 80] = aux[:, :, A_WQ:A_WQ + 80]
    # resolve the atan range-fix branch on host: the sign of
    # (atan(q) - atan(gw/gh)) flips under q -> 1/q reflection but the
    # square is invariant, so upload atg or pi/2-atg per slot
    x2 = posc2[:, :, P_BOX + 12:P_BOX + 18].astype(np.float64)
    x3 = posc2[:, :, P_BOX + 18:P_BOX + 24].astype(np.float64)
    aw = aux[:, :, A_AWH:A_AWH + 6].astype(np.float64)
    ah = aux[:, :, A_AWH + 6:A_AWH + 12].astype(np.float64)
    w = x2 + np.log(aw) - x3 - np.log(ah)
    atg = aux[:, :, A_ATG:A_ATG + 6].astype(np.float64)
    aux3[:, :, B_ATGX:B_ATGX + 6] = np.where(
        w > 0, np.pi / 2 - atg, atg).astype(np.float32)
    big = np.concatenate([posc2[:, :, 0:P_SEL + 12], ch4], axis=2)
    big = np.ascontiguousarray(big.astype(ml_dtypes.bfloat16))
    return aux3, big, n_targets, n_pos


def kernel(p_raw, labels, label_mask, cls_weight):
    global LAST_RESULT
    if MODE.startswith("v3"):
        aux3, big, n_targets, n_pos = _host_prepare_v3(
            p_raw, labels, label_mask, cls_weight)
        in_maps = [{"aux": aux3[c], "big": big[c]} for c in range(NCORES)]
    else:
        ch4, posc2, aux, n_targets, n_pos = _host_prepare(
            p_raw, labels, label_mask, cls_weight)
        in_maps = [
            {"ch4": ch4[c], "posc2": posc2[c], "aux": aux[c]}
            for c in range(NCORES)
        ]

    if MODE not in _BUILD_CACHE:
        _BUILD_CACHE[MODE] = _build(MODE)
    nc = _BUILD_CACHE[MODE]
    r = run_bass_kernel_spmd(
        nc, in_maps, core_ids=list(range(NCORES)), trace=TRACE, **TRACE_KW
    )
    LAST_RESULT = r

    outs = np.stack([np.asarray(r.results[c]["out"]) for c in range(NCORES)])
    s = outs.astype(np.float64).sum(axis=(0, 1))
    total = (7.5 * (n_targets + s[COL_BOX]) / n_pos
             + 0.25 / NTOT * s[COL_OBJ]
             + 0.125 / (n_pos * C) * s[COL_CLS]
             + s[COL_CORR])
    return np.float32(total)


# revision 12
# speedup vs baseline: 1.2156x; 1.2156x over previous
"""Trainium2 Bass kernel for nn_DBLoss (YOLO-style detection loss).

Strategy (pure data parallel over batch, 8 cores x 4 images):
  * Loss = 7.5*l_box + l_obj + 0.5*l_cls.  Only the objectness term
    touches every grid cell; box/cls touch only the <=720 label-assigned
    cells per core.
  * Host (numpy) replicates the reference's target assignment on the tiny
    `labels` tensor (as in the original baseline) and builds per-core
    device inputs during sharding:
      - ch4   [128,600]  objectness logits, contiguous (one fast DMA
               instead of 70k strided 4B descriptors -- the old bottleneck)
      - posc2 [128,516]  positive-cell cls logits (class-major), selected
               correction logits, and box logits (quantity-major)
      - aux   [128,170]  per-slot box constants, correction weights,
               cls_weight
  * Device computes ALL loss math:
      - dense focal_bce(x,0) over all 76800 cells/core via ACT exp/ln
        (f0 = sigmoid^1.5 * softplus = exp(1.5*(x-l))*l, l=softplus(x))
      - the same f0 form for the 80-class focal loss at positive cells
      - a t=0 -> t=1 correction at positive (cell,channel) pairs
        (f1-f0 = exp(-1.5l)*(l-x) - exp(1.5(x-l))*l), covering both the
        objectness targets and the one-hot class targets in one pass
      - CIoU box loss on [128,12] x|y-packed tiles split across DVE+Pool,
        atan via a degree-7 odd polynomial, reciprocals via the 1-op
        approx-NR custom DVE op
      - per-partition partial sums via fused accum_out reductions
  * Host sums 8x128x4 partials (f64) and applies the loss weights and
    n_pos / mean normalizations.
"""

import sys

sys.path.insert(0, "/opt/trn_rl_repo")

import numpy as np

import concourse.bass as bass
import concourse.tile as tile
from concourse import mybir
from concourse.bass_utils import run_bass_kernel_spmd

f32 = mybir.dt.float32
AF = mybir.ActivationFunctionType
ALU = mybir.AluOpType
AX = mybir.AxisListType

# problem constants (hardcoded per harness contract)
B, NA, H, W, M, C = 32, 3, 80, 80, 20, 80
NCORES = 8
BL = B // NCORES                 # 4 images per core
NCELL = BL * NA * H * W          # 76800 cells per core
KD = NCELL // 128                # 600 dense cols
NG = 6                           # positive-slot groups: 6*128 = 768 >= 720
NSEL = 12                        # correction entries: 12*128 = 1536 >= 1440
NTOT = B * NA * H * W            # 614400 cells globally
STRIDE = np.float32(8.0)
IMG = np.float32(640.0)
EPS = np.float32(1e-7)
PI2 = np.float32(np.pi ** 2)
ANCHORS = np.array([[10.0, 13.0], [16.0, 30.0], [33.0, 23.0]], dtype=np.float32)
EMPTY_CLS = np.float32(-30.0)    # cls logit filler: f0(-30) underflows to 0

# atan(z) ~ z*(A0 + A1 z^2 + A2 z^4 + A3 z^6) on [0,1], max abs err 1.5e-4
ATAN4 = [0.99874209, -0.31793283, 0.14020638, -0.03564737]

# aux column layout
A_CXY, A_AWH, A_G1, A_G2, A_GM = 0, 12, 24, 36, 48
A_AREA, A_ATG, A_VALID, A_SELW, A_WQ = 60, 66, 72, 78, 90
AUXW = 170
# posc2 column layout: [cls(480) | sel(12) | box logits(24)]
P_SEL, P_BOX = 480, 492
PCW = 516
# partials columns
COL_OBJ, COL_CLS, COL_CORR, COL_BOX, NCOL = 0, 1, 2, 3, 4

MODE = "v3"
TRACE = False
TRACE_KW = {}
LAST_RESULT = None
_BUILD_CACHE = {}


def _split_multi_waits(nc, limit=1):
    """This container's walrus build accepts only one sync-wait per
    instruction; split Tile's stacked waits into single-wait NoOp chains."""
    n = 0
    for fn in nc.m.functions:
        for bb in fn.blocks:
            new_insts, changed = [], False
            for inst in bb.instructions:
                si = getattr(inst, "sync_info", None)
                waits = list(si.on_wait) if si is not None and si.on_wait else []
                if len(waits) > limit:
                    changed = True
                    n += 1
                    for w in waits[:-limit]:
                        nop = mybir.InstNoOp(
                            name=nc.get_next_instruction_name(),
                            engine=inst.engine,
                            sync_info=mybir.SyncInfo(on_wait=[w], on_update=[]),
                            bass_nofuse=True,
                        )
                        nc.register_instruction(nop)
                        new_insts.append(nop)
                    si.on_wait = waits[-limit:]
                new_insts.append(inst)
            if changed:
                try:
                    bb.instructions = new_insts
                except Exception:
                    bb.instructions[:] = new_insts
    return n


def _acc_stt(nc, use_accum, out_t, in0, scalar, in1, acc_col):
    """out = (in0*scalar)*in1; acc_col[:,0] = row-sum, fused or 2-op."""
    if use_accum:
        nc.vector.scalar_tensor_tensor(
            out=out_t[:], in0=in0, scalar=float(scalar), in1=in1,
            op0=ALU.mult, op1=ALU.mult, accum_out=acc_col)
    else:
        nc.vector.scalar_tensor_tensor(
            out=out_t[:], in0=in0, scalar=float(scalar), in1=in1,
            op0=ALU.mult, op1=ALU.mult)
        nc.vector.tensor_reduce(out=acc_col, in_=out_t[:], axis=AX.X,
                                op=ALU.add)


def _build_v1(use_pool=True, use_accum=True):
    nc = bass.Bass()
    ch4 = nc.declare_dram_parameter("ch4", [128, KD], f32, isOutput=False)
    posc2 = nc.declare_dram_parameter("posc2", [128, PCW], f32, isOutput=False)
    aux = nc.declare_dram_parameter("aux", [128, AUXW], f32, isOutput=False)
    outp = nc.declare_dram_parameter("out", [128, NCOL], f32, isOutput=True)

    K_V = float(np.float32(4.0) / PI2)

    with tile.TileContext(nc) as tc:
        with tc.tile_pool(name="main", bufs=1) as pool:
            PE = nc.gpsimd if use_pool else nc.vector
            # ---- input DMAs, one per HWDGE ring, all issued at t=0 ----
            x_p = pool.tile([128, PCW], f32)         # cls+sel+box logits
            nc.scalar.dma_start(out=x_p[:], in_=posc2[:])
            x_a = pool.tile([128, AUXW], f32)        # constants
            nc.sync.dma_start(out=x_a[:], in_=aux[:])
            x_o = pool.tile([128, KD], f32)          # dense obj logits
            nc.sync.dma_start(out=x_o[:], in_=ch4[:])

            partials = pool.tile([128, NCOL], f32)

            def T(name, n):
                return pool.tile([128, n], f32, name=name)

            # aux views
            cxy = x_a[:, A_CXY:A_CXY + 12]
            awh = x_a[:, A_AWH:A_AWH + 12]
            g1 = x_a[:, A_G1:A_G1 + 12]
            g2 = x_a[:, A_G2:A_G2 + 12]
            gm = x_a[:, A_GM:A_GM + 12]
            areagE = x_a[:, A_AREA:A_AREA + 6]
            atg = x_a[:, A_ATG:A_ATG + 6]
            valid = x_a[:, A_VALID:A_VALID + 6]
            selw = x_a[:, A_SELW:A_SELW + 12]
            wq80 = x_a[:, A_WQ:A_WQ + 80]
            pos4 = x_p[:, P_BOX:PCW]                  # [x0|x1|x2|x3] blocks
            xcs = x_p[:, 0:P_SEL + 12]                # cls + sel logits

            # ============ ACT: box exps first (unblocks the long chain)
            e4 = T("e4", 24)
            nc.scalar.activation(e4[:], pos4, AF.Exp)

            # ============ DVE+Pool: CIoU box loss on x|y-packed [128,12]
            e2p1 = T("e2p1", 12)
            nc.vector.tensor_scalar_add(e2p1[:], e4[:, 0:12], 1.0)
            r2 = T("r2", 12)
            nc.vector.reciprocal(out=r2[:], in_=e2p1[:])
            pxy = T("pxy", 12)                        # center coords (px|py)
            nc.vector.scalar_tensor_tensor(
                out=pxy[:], in0=r2[:], scalar=-8.0, in1=cxy,
                op0=ALU.mult, op1=ALU.add)
            pwh = T("pwh", 12)                        # box sizes (pw|ph)
            PE.tensor_tensor(out=pwh[:], in0=e4[:, 12:24], in1=awh,
                                    op=ALU.mult)
            th = T("th", 12)
            PE.tensor_scalar_mul(th[:], pwh[:], 0.5)
            p1 = T("p1", 12)
            PE.tensor_tensor(out=p1[:], in0=pxy[:], in1=th[:],
                                    op=ALU.subtract)
            p2 = T("p2", 12)
            PE.tensor_tensor(out=p2[:], in0=pxy[:], in1=th[:],
                                    op=ALU.add)
            m1 = T("m1", 12)
            nc.vector.tensor_tensor(out=m1[:], in0=p2[:], in1=g2, op=ALU.min)
            m2 = T("m2", 12)
            nc.vector.tensor_tensor(out=m2[:], in0=p1[:], in1=g1, op=ALU.max)
            iwh = T("iwh", 12)
            PE.tensor_tensor(out=iwh[:], in0=m1[:], in1=m2[:],
                                    op=ALU.subtract)
            PE.tensor_scalar_max(iwh[:], iwh[:], 0.0)
            M1 = T("M1", 12)
            nc.vector.tensor_tensor(out=M1[:], in0=p2[:], in1=g2, op=ALU.max)
            M2 = T("M2", 12)
            nc.vector.tensor_tensor(out=M2[:], in0=p1[:], in1=g1, op=ALU.min)
            cwh = T("cwh", 12)
            PE.tensor_tensor(out=cwh[:], in0=M1[:], in1=M2[:],
                                    op=ALU.subtract)
            dd = T("dd", 12)
            PE.tensor_tensor(out=dd[:], in0=pxy[:], in1=gm,
                                    op=ALU.subtract)

            inter = T("inter", 6)
            nc.vector.tensor_tensor(out=inter[:], in0=iwh[:, 0:6],
                                    in1=iwh[:, 6:12], op=ALU.mult)
            areap = T("areap", 6)
            PE.tensor_tensor(out=areap[:], in0=pwh[:, 0:6],
                                    in1=pwh[:, 6:12], op=ALU.mult)
            union = T("union", 6)
            PE.tensor_tensor(out=union[:], in0=areap[:], in1=areagE,
                                    op=ALU.add)
            nc.vector.tensor_tensor(out=union[:], in0=union[:], in1=inter[:],
                                    op=ALU.subtract)
            runi = T("runi", 6)
            nc.vector.reciprocal(out=runi[:], in_=union[:])
            iou = T("iou", 6)
            nc.vector.tensor_tensor(out=iou[:], in0=inter[:], in1=runi[:],
                                    op=ALU.mult)

            csq = T("csq", 12)
            PE.tensor_tensor(out=csq[:], in0=cwh[:], in1=cwh[:],
                                    op=ALU.mult)
            c2e = T("c2e", 6)
            PE.tensor_tensor(out=c2e[:], in0=csq[:, 0:6],
                                    in1=csq[:, 6:12], op=ALU.add)
            PE.tensor_scalar_add(c2e[:], c2e[:], float(EPS))
            rc2 = T("rc2", 6)
            nc.vector.reciprocal(out=rc2[:], in_=c2e[:])
            dsq = T("dsq", 12)
            PE.tensor_tensor(out=dsq[:], in0=dd[:], in1=dd[:],
                                    op=ALU.mult)
            rho2 = T("rho2", 6)
            PE.tensor_tensor(out=rho2[:], in0=dsq[:, 0:6],
                                    in1=dsq[:, 6:12], op=ALU.add)
            rho2c2 = T("rho2c2", 6)
            nc.vector.tensor_tensor(out=rho2c2[:], in0=rho2[:], in1=rc2[:],
                                    op=ALU.mult)

            # v = 4/pi^2 * (atan(gw/gh) - atan(pw/ph))^2 via poly atan
            phe = T("phe", 6)
            nc.vector.tensor_scalar_add(phe[:], pwh[:, 6:12], float(EPS))
            rph = T("rph", 6)
            nc.vector.reciprocal(out=rph[:], in_=phe[:])
            q = T("q", 6)
            nc.vector.tensor_tensor(out=q[:], in0=pwh[:, 0:6], in1=rph[:],
                                    op=ALU.mult)
            rq = T("rq", 6)
            nc.vector.reciprocal(out=rq[:], in_=q[:])
            z = T("z", 6)
            nc.vector.tensor_tensor(out=z[:], in0=q[:], in1=rq[:], op=ALU.min)
            z2 = T("z2", 6)
            PE.tensor_tensor(out=z2[:], in0=z[:], in1=z[:], op=ALU.mult)
            acc = T("acc", 6)
            PE.tensor_scalar(
                out=acc[:], in0=z2[:], scalar1=float(ATAN4[3]),
                scalar2=float(ATAN4[2]), op0=ALU.mult, op1=ALU.add)
            PE.tensor_tensor(out=acc[:], in0=acc[:], in1=z2[:],
                                    op=ALU.mult)
            PE.tensor_scalar_add(acc[:], acc[:], float(ATAN4[1]))
            PE.tensor_tensor(out=acc[:], in0=acc[:], in1=z2[:],
                                    op=ALU.mult)
            PE.tensor_scalar_add(acc[:], acc[:], float(ATAN4[0]))
            at0 = T("at0", 6)
            PE.tensor_tensor(out=at0[:], in0=acc[:], in1=z[:],
                                    op=ALU.mult)
            # range fix: at = at0 + (q>1)*(pi/2 - 2*at0)
            flag = T("flag", 6)
            nc.vector.tensor_scalar(
                out=flag[:], in0=q[:], scalar1=1.0, scalar2=None, op0=ALU.is_gt)
            fw = T("fw", 6)
            PE.tensor_scalar(
                out=fw[:], in0=at0[:], scalar1=-2.0,
                scalar2=float(np.pi / 2), op0=ALU.mult, op1=ALU.add)
            PE.tensor_tensor(out=fw[:], in0=fw[:], in1=flag[:],
                                    op=ALU.mult)
            at = T("at", 6)
            PE.tensor_tensor(out=at[:], in0=at0[:], in1=fw[:],
                                    op=ALU.add)
            dv = T("dv", 6)
            PE.tensor_tensor(out=dv[:], in0=atg, in1=at[:],
                                    op=ALU.subtract)
            v = T("v", 6)
            PE.tensor_tensor(out=v[:], in0=dv[:], in1=dv[:],
                                    op=ALU.mult)
            PE.tensor_scalar_mul(v[:], v[:], K_V)
            den = T("den", 6)
            nc.vector.scalar_tensor_tensor(
                out=den[:], in0=iou[:], scalar=-1.0, in1=v[:],
                op0=ALU.mult, op1=ALU.add)
            nc.vector.tensor_scalar_add(den[:], den[:], float(1.0 + float(EPS)))
            rden = T("rden", 6)
            nc.vector.reciprocal(out=rden[:], in_=den[:])
            av = T("av", 6)
            nc.vector.tensor_tensor(out=av[:], in0=v[:], in1=rden[:],
                                    op=ALU.mult)
            nc.vector.tensor_tensor(out=av[:], in0=av[:], in1=v[:],
                                    op=ALU.mult)
            li = T("li", 6)
            PE.tensor_tensor(out=li[:], in0=av[:], in1=rho2c2[:],
                                    op=ALU.add)
            nc.vector.tensor_tensor(out=li[:], in0=li[:], in1=iou[:],
                                    op=ALU.subtract)
            # per-slot loss = 1 + li; the +1*n_pos is added on host
            jb = T("jb", 6)
            _acc_stt(nc, use_accum, jb, li[:], 1.0, valid,
                     partials[:, COL_BOX:COL_BOX + 1])

            # ============ ACT/DVE: f0 = exp(1.5*(x-l))*l pipelines
            # cls+sel block [128,492]
            e_cs = T("e_cs", P_SEL + 12)
            nc.scalar.activation(e_cs[:], xcs, AF.Exp)
            l_cs = T("l_cs", P_SEL + 12)
            nc.scalar.activation(l_cs[:], e_cs[:], AF.Ln, bias=1.0)
            d_cs = T("d_cs", P_SEL + 12)
            nc.vector.tensor_tensor(out=d_cs[:], in0=xcs, in1=l_cs[:],
                                    op=ALU.subtract)
            # dense obj block [128,600]
            e_o = T("e_o", KD)
            nc.scalar.activation(e_o[:], x_o[:], AF.Exp)
            l_o = T("l_o", KD)
            nc.scalar.activation(l_o[:], e_o[:], AF.Ln, bias=1.0)
            d_o = T("d_o", KD)
            nc.vector.tensor_tensor(out=d_o[:], in0=x_o[:], in1=l_o[:],
                                    op=ALU.subtract)
            u_cs = T("u_cs", P_SEL + 12)
            nc.scalar.activation(u_cs[:], d_cs[:], AF.Exp, scale=1.5)
            u_o = T("u_o", KD)
            nc.scalar.activation(u_o[:], d_o[:], AF.Exp, scale=1.5)
            h1 = T("h1", 12)
            nc.scalar.activation(h1[:], l_cs[:, P_SEL:P_SEL + 12], AF.Exp,
                                 scale=-1.5)

            # dense obj: sum f0 = sum u*l
            jo = T("jo", KD)
            _acc_stt(nc, use_accum, jo, u_o[:], 1.0, l_o[:],
                     partials[:, COL_OBJ:COL_OBJ + 1])

            # cls + sel f0 products
            P_cs = T("P_cs", P_SEL + 12)
            nc.vector.tensor_tensor(out=P_cs[:], in0=u_cs[:], in1=l_cs[:],
                                    op=ALU.mult)
            # cls: reduce slots (class-major layout -> innermost g), then *w
            red80 = T("red80", 80)
            nc.vector.tensor_reduce(
                out=red80[:], in_=P_cs[:, 0:P_SEL].rearrange(
                    "p (c g) -> p c g", g=NG),
                axis=AX.X, op=ALU.add)
            j80 = T("j80", 80)
            _acc_stt(nc, use_accum, j80, red80[:], 1.0, wq80,
                     partials[:, COL_CLS:COL_CLS + 1])

            # corr: f1 - f0 = h1*(l-x) - P  at selected (cell,ch) pairs
            f1n = T("f1n", 12)
            PE.tensor_tensor(out=f1n[:], in0=h1[:],
                                    in1=d_cs[:, P_SEL:P_SEL + 12],
                                    op=ALU.mult)
            ncor = T("ncor", 12)
            PE.tensor_tensor(out=ncor[:], in0=f1n[:],
                                    in1=P_cs[:, P_SEL:P_SEL + 12],
                                    op=ALU.add)
            jc = T("jc", 12)
            _acc_stt(nc, use_accum, jc, ncor[:], -1.0, selw,
                     partials[:, COL_CORR:COL_CORR + 1])

            # ---- store per-partition partials; host reduces across cores
            nc.sync.dma_start(out=outp[:], in_=partials[:])

    _split_multi_waits(nc)
    return nc




def _build_v2():
    """All-DVE box chain with fused/packed ops; Pool runs only the atan
    polynomial and corr product branches; all bulk DMAs on the ACT ring
    (the sync-ring DMA queue is packet-rate-limited ~25M pkt/s)."""
    nc = bass.Bass()
    ch4 = nc.declare_dram_parameter("ch4", [128, KD], f32, isOutput=False)
    posc2 = nc.declare_dram_parameter("posc2", [128, PCW], f32, isOutput=False)
    aux = nc.declare_dram_parameter("aux", [128, AUXW], f32, isOutput=False)
    outp = nc.declare_dram_parameter("out", [128, NCOL], f32, isOutput=True)

    K_V = float(np.float32(4.0) / PI2)

    with tile.TileContext(nc) as tc:
        with tc.tile_pool(name="main", bufs=1) as pool:
            x_p = pool.tile([128, PCW], f32)
            nc.scalar.dma_start(out=x_p[:], in_=posc2[:])
            x_a = pool.tile([128, AUXW], f32)
            nc.scalar.dma_start(out=x_a[:], in_=aux[:])
            x_o = pool.tile([128, KD], f32)
            nc.scalar.dma_start(out=x_o[:], in_=ch4[:])

            partials = pool.tile([128, NCOL], f32)

            def T(name, n):
                return pool.tile([128, n], f32, name=name)

            cxy = x_a[:, A_CXY:A_CXY + 12]
            awh = x_a[:, A_AWH:A_AWH + 12]
            g1 = x_a[:, A_G1:A_G1 + 12]
            g2 = x_a[:, A_G2:A_G2 + 12]
            gm = x_a[:, A_GM:A_GM + 12]
            areagE = x_a[:, A_AREA:A_AREA + 6]
            atg = x_a[:, A_ATG:A_ATG + 6]
            valid = x_a[:, A_VALID:A_VALID + 6]
            selw = x_a[:, A_SELW:A_SELW + 12]
            wq80 = x_a[:, A_WQ:A_WQ + 80]
            pos4 = x_p[:, P_BOX:PCW]
            xcs = x_p[:, 0:P_SEL + 12]

            # ============ ACT: box exps first
            e4 = T("e4", 24)
            nc.scalar.activation(e4[:], pos4, AF.Exp)

            # ============ DVE box chain (x|y packed [128,12])
            e2p1 = T("e2p1", 12)
            nc.vector.tensor_scalar_add(e2p1[:], e4[:, 0:12], 1.0)
            r2 = T("r2", 12)
            nc.vector.reciprocal(out=r2[:], in_=e2p1[:])
            pxy = T("pxy", 12)
            nc.vector.scalar_tensor_tensor(
                out=pxy[:], in0=r2[:], scalar=-8.0, in1=cxy,
                op0=ALU.mult, op1=ALU.add)
            pwh = T("pwh", 12)
            nc.vector.tensor_tensor(out=pwh[:], in0=e4[:, 12:24], in1=awh,
                                    op=ALU.mult)
            th = T("th", 12)
            nc.vector.tensor_scalar_mul(th[:], pwh[:], 0.5)
            p1 = T("p1", 12)
            nc.vector.tensor_tensor(out=p1[:], in0=pxy[:], in1=th[:],
                                    op=ALU.subtract)
            p2 = T("p2", 12)
            nc.vector.tensor_tensor(out=p2[:], in0=pxy[:], in1=th[:],
                                    op=ALU.add)
            # rwh = 1/pwh for both q and qi (ph,pw >= 0.03 always; no EPS)
            rwh = T("rwh", 12)
            nc.vector.reciprocal(out=rwh[:], in_=pwh[:])
            # packed [min|max] pairs -> one subtract gives [iw_raw | cw]
            mM1 = T("mM1", 24)
            nc.vector.tensor_tensor(out=mM1[:, 0:12], in0=p2[:], in1=g2,
                                    op=ALU.min)
            nc.vector.tensor_tensor(out=mM1[:, 12:24], in0=p2[:], in1=g2,
                                    op=ALU.max)
            mM2 = T("mM2", 24)
            nc.vector.tensor_tensor(out=mM2[:, 0:12], in0=p1[:], in1=g1,
                                    op=ALU.max)
            nc.vector.tensor_tensor(out=mM2[:, 12:24], in0=p1[:], in1=g1,
                                    op=ALU.min)
            dif = T("dif", 24)
            nc.vector.tensor_tensor(out=dif[:], in0=mM1[:], in1=mM2[:],
                                    op=ALU.subtract)
            iwh = T("iwh", 12)
            nc.vector.tensor_scalar_max(iwh[:], dif[:, 0:12], 0.0)
            # Pool branch A: q/z/atan polynomial (independent after rwh/pwh)
            q6 = T("q6", 12)                     # [q | qi]
            nc.gpsimd.tensor_tensor(out=q6[:, 0:6], in0=pwh[:, 0:6],
                                    in1=rwh[:, 6:12], op=ALU.mult)
            nc.gpsimd.tensor_tensor(out=q6[:, 6:12], in0=pwh[:, 6:12],
                                    in1=rwh[:, 0:6], op=ALU.mult)
            z = T("z", 6)
            nc.vector.tensor_tensor(out=z[:], in0=q6[:, 0:6], in1=q6[:, 6:12],
                                    op=ALU.min)
            z2 = T("z2", 6)
            nc.gpsimd.tensor_tensor(out=z2[:], in0=z[:], in1=z[:],
                                    op=ALU.mult)
            acc = T("acc", 6)
            nc.gpsimd.tensor_scalar(
                out=acc[:], in0=z2[:], scalar1=float(ATAN4[3]),
                scalar2=float(ATAN4[2]), op0=ALU.mult, op1=ALU.add)
            nc.gpsimd.tensor_tensor(out=acc[:], in0=acc[:], in1=z2[:],
                                    op=ALU.mult)
            nc.gpsimd.tensor_scalar_add(acc[:], acc[:], float(ATAN4[1]))
            nc.gpsimd.tensor_tensor(out=acc[:], in0=acc[:], in1=z2[:],
                                    op=ALU.mult)
            nc.gpsimd.tensor_scalar_add(acc[:], acc[:], float(ATAN4[0]))
            at0 = T("at0", 6)
            nc.gpsimd.tensor_tensor(out=at0[:], in0=acc[:], in1=z[:],
                                    op=ALU.mult)
            flag = T("flag", 6)
            nc.gpsimd.tensor_scalar(
                out=flag[:], in0=q6[:, 0:6], scalar1=1.0, scalar2=None,
                op0=ALU.is_gt)
            fw = T("fw", 6)
            nc.gpsimd.tensor_scalar(
                out=fw[:], in0=at0[:], scalar1=-2.0,
                scalar2=float(np.pi / 2), op0=ALU.mult, op1=ALU.add)
            nc.gpsimd.tensor_tensor(out=fw[:], in0=fw[:], in1=flag[:],
                                    op=ALU.mult)
            at = T("at", 6)
            nc.gpsimd.tensor_tensor(out=at[:], in0=at0[:], in1=fw[:],
                                    op=ALU.add)
            dv = T("dv", 6)
            nc.gpsimd.tensor_tensor(out=dv[:], in0=atg, in1=at[:],
                                    op=ALU.subtract)
            v = T("v", 6)
            nc.gpsimd.tensor_tensor(out=v[:], in0=dv[:], in1=dv[:],
                                    op=ALU.mult)
            nc.gpsimd.tensor_scalar_mul(v[:], v[:], K_V)
            # DVE main: inter/union/c2/rho2
            inter = T("inter", 6)
            nc.vector.tensor_tensor(out=inter[:], in0=iwh[:, 0:6],
                                    in1=iwh[:, 6:12], op=ALU.mult)
            areap = T("areap", 6)
            nc.vector.tensor_tensor(out=areap[:], in0=pwh[:, 0:6],
                                    in1=pwh[:, 6:12], op=ALU.mult)
            ucb = T("ucb", 12)                   # [union | c2]
            nc.vector.tensor_tensor(out=ucb[:, 0:6], in0=areap[:],
                                    in1=areagE, op=ALU.add)
            nc.vector.tensor_tensor(out=ucb[:, 0:6], in0=ucb[:, 0:6],
                                    in1=inter[:], op=ALU.subtract)
            csq = T("csq", 12)
            nc.vector.tensor_tensor(out=csq[:], in0=dif[:, 12:24],
                                    in1=dif[:, 12:24], op=ALU.mult)
            nc.vector.tensor_tensor(out=ucb[:, 6:12], in0=csq[:, 0:6],
                                    in1=csq[:, 6:12], op=ALU.add)
            rb = T("rb", 12)                     # [1/union | 1/c2]
            nc.vector.reciprocal(out=rb[:], in_=ucb[:])
            iou = T("iou", 6)
            nc.vector.tensor_tensor(out=iou[:], in0=inter[:], in1=rb[:, 0:6],
                                    op=ALU.mult)
            dd = T("dd", 12)
            nc.vector.tensor_tensor(out=dd[:], in0=pxy[:], in1=gm,
                                    op=ALU.subtract)
            dsq = T("dsq", 12)
            nc.vector.tensor_tensor(out=dsq[:], in0=dd[:], in1=dd[:],
                                    op=ALU.mult)
            rho2 = T("rho2", 6)
            nc.vector.tensor_tensor(out=rho2[:], in0=dsq[:, 0:6],
                                    in1=dsq[:, 6:12], op=ALU.add)
            rho2c2 = T("rho2c2", 6)
            nc.vector.tensor_tensor(out=rho2c2[:], in0=rho2[:],
                                    in1=rb[:, 6:12], op=ALU.mult)
            den = T("den", 6)
            nc.vector.scalar_tensor_tensor(
                out=den[:], in0=iou[:], scalar=-1.0, in1=v[:],
                op0=ALU.mult, op1=ALU.add)
            nc.vector.tensor_scalar_add(den[:], den[:], float(1.0 + float(EPS)))
            rden = T("rden", 6)
            nc.vector.reciprocal(out=rden[:], in_=den[:])
            av = T("av", 6)
            nc.vector.tensor_tensor(out=av[:], in0=v[:], in1=rden[:],
                                    op=ALU.mult)
            nc.vector.tensor_tensor(out=av[:], in0=av[:], in1=v[:],
                                    op=ALU.mult)
            li = T("li", 6)
            nc.vector.tensor_tensor(out=li[:], in0=av[:], in1=rho2c2[:],
                                    op=ALU.add)
            nc.vector.tensor_tensor(out=li[:], in0=li[:], in1=iou[:],
                                    op=ALU.subtract)
            jb = T("jb", 6)
            nc.vector.scalar_tensor_tensor(
                out=jb[:], in0=li[:], scalar=1.0, in1=valid,
                op0=ALU.mult, op1=ALU.mult)
            nc.vector.tensor_reduce(
                out=partials[:, COL_BOX:COL_BOX + 1], in_=jb[:], axis=AX.X,
                op=ALU.add)

            # ============ f0 pipelines (ACT exp/ln + DVE)
            e_cs = T("e_cs", P_SEL + 12)
            nc.scalar.activation(e_cs[:], xcs, AF.Exp)
            l_cs = T("l_cs", P_SEL + 12)
            nc.scalar.activation(l_cs[:], e_cs[:], AF.Ln, bias=1.0)
            d_cs = T("d_cs", P_SEL + 12)
            nc.vector.tensor_tensor(out=d_cs[:], in0=xcs, in1=l_cs[:],
                                    op=ALU.subtract)
            e_o = T("e_o", KD)
            nc.scalar.activation(e_o[:], x_o[:], AF.Exp)
            l_o = T("l_o", KD)
            nc.scalar.activation(l_o[:], e_o[:], AF.Ln, bias=1.0)
            d_o = T("d_o", KD)
            nc.vector.tensor_tensor(out=d_o[:], in0=x_o[:], in1=l_o[:],
                                    op=ALU.subtract)
            u_cs = T("u_cs", P_SEL + 12)
            nc.scalar.activation(u_cs[:], d_cs[:], AF.Exp, scale=1.5)
            u_o = T("u_o", KD)
            nc.scalar.activation(u_o[:], d_o[:], AF.Exp, scale=1.5)
            h1 = T("h1", 12)
            nc.scalar.activation(h1[:], l_cs[:, P_SEL:P_SEL + 12], AF.Exp,
                                 scale=-1.5)

            jo = T("jo", KD)
            nc.vector.tensor_tensor(out=jo[:], in0=u_o[:], in1=l_o[:],
                                    op=ALU.mult)
            nc.vector.tensor_reduce(
                out=partials[:, COL_OBJ:COL_OBJ + 1], in_=jo[:], axis=AX.X,
                op=ALU.add)

            P_cs = T("P_cs", P_SEL + 12)
            nc.vector.tensor_tensor(out=P_cs[:], in0=u_cs[:], in1=l_cs[:],
                                    op=ALU.mult)
            red80 = T("red80", 80)
            nc.vector.tensor_reduce(
                out=red80[:], in_=P_cs[:, 0:P_SEL].rearrange(
                    "p (c g) -> p c g", g=NG),
                axis=AX.X, op=ALU.add)
            j80 = T("j80", 80)
            nc.vector.tensor_tensor(out=j80[:], in0=red80[:], in1=wq80,
                                    op=ALU.mult)
            nc.vector.tensor_reduce(
                out=partials[:, COL_CLS:COL_CLS + 1], in_=j80[:], axis=AX.X,
                op=ALU.add)

            # corr on Pool (2 ops), final weighted reduce on DVE
            f1n = T("f1n", 12)
            nc.gpsimd.tensor_tensor(out=f1n[:], in0=h1[:],
                                    in1=d_cs[:, P_SEL:P_SEL + 12],
                                    op=ALU.mult)
            ncor = T("ncor", 12)
            nc.gpsimd.tensor_tensor(out=ncor[:], in0=f1n[:],
                                    in1=P_cs[:, P_SEL:P_SEL + 12],
                                    op=ALU.add)
            jc = T("jc", 12)
            nc.vector.scalar_tensor_tensor(
                out=jc[:], in0=ncor[:], scalar=-1.0, in1=selw,
                op0=ALU.mult, op1=ALU.mult)
            nc.vector.tensor_reduce(
                out=partials[:, COL_CORR:COL_CORR + 1], in_=jc[:], axis=AX.X,
                op=ALU.add)

            nc.sync.dma_start(out=outp[:], in_=partials[:])

    _split_multi_waits(nc)
    return nc




# V3 aux layout (f32)
B_POS4, B_CXY, B_AWH, B_G1, B_G2, B_GM = 0, 24, 36, 48, 60, 72
B_AREA, B_ATGX, B_VALID, B_SELW, B_WQ = 84, 90, 96, 102, 114
AUX3 = 194
# big (bf16): [cls(480) | sel(12) | ch4(600)]
BIGW = 1092
bf16 = mybir.dt.bfloat16
# atan deg-5 odd poly on [0,1], max err 1.0e-3
ATAN5 = [0.9931425, -0.28070902, 0.07320315]


def _build_v3():
    """bf16 data path, merged exp/ln/u mega-ops, host-selected atan branch
    (no flag ops), fused squares, aux-first DMA so the box chain starts
    as early as possible."""
    nc = bass.Bass()
    aux = nc.declare_dram_parameter("aux", [128, AUX3], f32, isOutput=False)
    big = nc.declare_dram_parameter("big", [128, BIGW], bf16, isOutput=False)
    outp = nc.declare_dram_parameter("out", [128, NCOL], f32, isOutput=True)

    K_V = float(np.float32(4.0) / PI2)

    with tile.TileContext(nc) as tc:
        with tc.tile_pool(name="main", bufs=1) as pool:
            x_a = pool.tile([128, AUX3], f32)
            nc.scalar.dma_start(out=x_a[:], in_=aux[:])
            x_b = pool.tile([128, BIGW], bf16)
            nc.scalar.dma_start(out=x_b[:], in_=big[:])

            partials = pool.tile([128, NCOL], f32)

            def T(name, n, dt=f32):
                return pool.tile([128, n], dt, name=name)

            pos4 = x_a[:, B_POS4:B_POS4 + 24]
            cxy = x_a[:, B_CXY:B_CXY + 12]
            awh = x_a[:, B_AWH:B_AWH + 12]
            g1 = x_a[:, B_G1:B_G1 + 12]
            g2 = x_a[:, B_G2:B_G2 + 12]
            gm = x_a[:, B_GM:B_GM + 12]
            areagE = x_a[:, B_AREA:B_AREA + 6]
            atgx = x_a[:, B_ATGX:B_ATGX + 6]
            valid = x_a[:, B_VALID:B_VALID + 6]
            selw = x_a[:, B_SELW:B_SELW + 12]
            wq80 = x_a[:, B_WQ:B_WQ + 80]

            # ---- ACT: box exps + (e4+1) for the sigmoid reciprocals
            e4 = T("e4", 24)
            nc.scalar.activation(e4[:], pos4, AF.Exp)
            e2p1 = T("e2p1", 12)
            nc.scalar.activation(e2p1[:], e4[:, 0:12], AF.Identity, bias=1.0)

            # ---- DVE box chain
            r2 = T("r2", 12)
            nc.vector.reciprocal(out=r2[:], in_=e2p1[:])
            pxy = T("pxy", 12)
            nc.vector.scalar_tensor_tensor(
                out=pxy[:], in0=r2[:], scalar=-8.0, in1=cxy,
                op0=ALU.mult, op1=ALU.add)
            pwh = T("pwh", 12)
            nc.vector.tensor_tensor(out=pwh[:], in0=e4[:, 12:24], in1=awh,
                                    op=ALU.mult)
            th = T("th", 12)
            nc.vector.tensor_scalar_mul(th[:], pwh[:], 0.5)
            p1 = T("p1", 12)
            nc.vector.tensor_tensor(out=p1[:], in0=pxy[:], in1=th[:],
                                    op=ALU.subtract)
            p2 = T("p2", 12)
            nc.vector.tensor_tensor(out=p2[:], in0=pxy[:], in1=th[:],
                                    op=ALU.add)
            mM1 = T("mM1", 24)
            nc.vector.tensor_tensor(out=mM1[:, 0:12], in0=p2[:], in1=g2,
                                    op=ALU.min)
            nc.vector.tensor_tensor(out=mM1[:, 12:24], in0=p2[:], in1=g2,
                                    op=ALU.max)
            mM2 = T("mM2", 24)
            nc.vector.tensor_tensor(out=mM2[:, 0:12], in0=p1[:], in1=g1,
                                    op=ALU.max)
            nc.vector.tensor_tensor(out=mM2[:, 12:24], in0=p1[:], in1=g1,
                                    op=ALU.min)
            # sqin = [iw_raw | cw | dd]; one 36-wide square covers all
            sqin = T("sqin", 36)
            nc.vector.tensor_tensor(out=sqin[:, 0:24], in0=mM1[:],
                                    in1=mM2[:], op=ALU.subtract)
            nc.vector.tensor_tensor(out=sqin[:, 24:36], in0=pxy[:], in1=gm,
                                    op=ALU.subtract)
            sqv = T("sqv", 36)
            nc.vector.tensor_tensor(out=sqv[:, 12:36], in0=sqin[:, 12:36],
                                    in1=sqin[:, 12:36], op=ALU.mult)
            iwh = T("iwh", 12)
            nc.vector.tensor_scalar_max(iwh[:], sqin[:, 0:12], 0.0)
            inter = T("inter", 6)
            nc.vector.tensor_tensor(out=inter[:], in0=iwh[:, 0:6],
                                    in1=iwh[:, 6:12], op=ALU.mult)
            areap = T("areap", 6)
            nc.vector.tensor_tensor(out=areap[:], in0=pwh[:, 0:6],
                                    in1=pwh[:, 6:12], op=ALU.mult)
            ucb = T("ucb", 12)
            nc.vector.tensor_tensor(out=ucb[:, 0:6], in0=areap[:],
                                    in1=areagE, op=ALU.add)
            nc.vector.tensor_tensor(out=ucb[:, 0:6], in0=ucb[:, 0:6],
                                    in1=inter[:], op=ALU.subtract)
            nc.vector.tensor_tensor(out=ucb[:, 6:12], in0=sqv[:, 12:18],
                                    in1=sqv[:, 18:24], op=ALU.add)
            rb = T("rb", 12)
            nc.vector.reciprocal(out=rb[:], in_=ucb[:])
            iou = T("iou", 6)
            nc.vector.tensor_tensor(out=iou[:], in0=inter[:], in1=rb[:, 0:6],
                                    op=ALU.mult)
            rho2 = T("rho2", 6)
            nc.vector.tensor_tensor(out=rho2[:], in0=sqv[:, 24:30],
                                    in1=sqv[:, 30:36], op=ALU.add)
            rho2c2 = T("rho2c2", 6)
            nc.vector.tensor_tensor(out=rho2c2[:], in0=rho2[:],
                                    in1=rb[:, 6:12], op=ALU.mult)
            # v branch: z = min(q, 1/q); q = pw/ph (pw,ph >= 0.03, no EPS)
            rwh = T("rwh", 12)
            nc.vector.reciprocal(out=rwh[:], in_=pwh[:])
            q6 = T("q6", 12)
            nc.vector.tensor_tensor(out=q6[:, 0:6], in0=pwh[:, 0:6],
                                    in1=rwh[:, 6:12], op=ALU.mult)
            nc.vector.tensor_tensor(out=q6[:, 6:12], in0=pwh[:, 6:12],
                                    in1=rwh[:, 0:6], op=ALU.mult)
            z = T("z", 6)
            nc.vector.tensor_tensor(out=z[:], in0=q6[:, 0:6], in1=q6[:, 6:12],
                                    op=ALU.min)
            # Pool: z2 + odd poly -> at0 = atan(z)
            z2 = T("z2", 6)
            nc.gpsimd.tensor_tensor(out=z2[:], in0=z[:], in1=z[:],
                                    op=ALU.mult)
            acc = T("acc", 6)
            nc.gpsimd.tensor_scalar(
                out=acc[:], in0=z2[:], scalar1=float(ATAN5[2]),
                scalar2=float(ATAN5[1]), op0=ALU.mult, op1=ALU.add)
            nc.gpsimd.tensor_tensor(out=acc[:], in0=acc[:], in1=z2[:],
                                    op=ALU.mult)
            nc.gpsimd.tensor_scalar_add(acc[:], acc[:], float(ATAN5[0]))
            at0 = T("at0", 6)
            nc.gpsimd.tensor_tensor(out=at0[:], in0=acc[:], in1=z[:],
                                    op=ALU.mult)
            # host pre-selected target angle (atg or pi/2-atg): sign of the
            # difference cancels in the square, so no range-fix ops needed
            dvx = T("dvx", 6)
            nc.vector.tensor_tensor(out=dvx[:], in0=at0[:], in1=atgx,
                                    op=ALU.subtract)
            vsq = T("vsq", 6)
            nc.vector.tensor_tensor(out=vsq[:], in0=dvx[:], in1=dvx[:],
                                    op=ALU.mult)
            vp1 = T("vp1", 6)
            nc.vector.tensor_scalar(
                out=vp1[:], in0=vsq[:], scalar1=K_V,
                scalar2=float(1.0 + float(EPS)), op0=ALU.mult, op1=ALU.add)
            v2k = T("v2k", 6)
            nc.vector.tensor_tensor(out=v2k[:], in0=vsq[:], in1=vsq[:],
                                    op=ALU.mult)
            den = T("den", 6)
            nc.vector.scalar_tensor_tensor(
                out=den[:], in0=iou[:], scalar=-1.0, in1=vp1[:],
                op0=ALU.mult, op1=ALU.add)
            rden = T("rden", 6)
            nc.vector.reciprocal(out=rden[:], in_=den[:])
            av = T("av", 6)
            nc.vector.scalar_tensor_tensor(
                out=av[:], in0=v2k[:], scalar=float(K_V * K_V), in1=rden[:],
                op0=ALU.mult, op1=ALU.mult)
            li = T("li", 6)
            nc.vector.tensor_tensor(out=li[:], in0=av[:], in1=rho2c2[:],
                                    op=ALU.add)
            nc.vector.tensor_tensor(out=li[:], in0=li[:], in1=iou[:],
                                    op=ALU.subtract)
            jb = T("jb", 6)
            nc.vector.scalar_tensor_tensor(
                out=jb[:], in0=li[:], scalar=1.0, in1=valid,
                op0=ALU.mult, op1=ALU.mult)
            nc.vector.tensor_reduce(
                out=partials[:, COL_BOX:COL_BOX + 1], in_=jb[:], axis=AX.X,
                op=ALU.add)

            # ---- merged f0 pipeline over [cls|sel|ch4] (bf16)
            e_all = T("e_all", BIGW, bf16)
            nc.scalar.activation(e_all[:], x_b[:], AF.Exp)
            l_all = T("l_all", BIGW, bf16)
            nc.scalar.activation(l_all[:], e_all[:], AF.Ln, bias=1.0)
            d_all = T("d_all", BIGW, bf16)
            nc.vector.tensor_tensor(out=d_all[:], in0=x_b[:], in1=l_all[:],
                                    op=ALU.subtract)
            u_all = T("u_all", BIGW, bf16)
            nc.scalar.activation(u_all[:], d_all[:], AF.Exp, scale=1.5)
            h1 = T("h1", 12, bf16)
            nc.scalar.activation(h1[:], l_all[:, P_SEL:P_SEL + 12], AF.Exp,
                                 scale=-1.5)
            P_all = T("P_all", BIGW, bf16)
            nc.vector.tensor_tensor(out=P_all[:], in0=u_all[:], in1=l_all[:],
                                    op=ALU.mult)
            # dense obj = sum over ch4 block
            nc.vector.tensor_reduce(
                out=partials[:, COL_OBJ:COL_OBJ + 1],
                in_=P_all[:, P_SEL + 12:BIGW], axis=AX.X, op=ALU.add)
            # cls: reduce slots (class-major, g innermost), then * weights
            red80 = T("red80", 80)
            nc.vector.tensor_reduce(
                out=red80[:], in_=P_all[:, 0:P_SEL].rearrange(
                    "p (c g) -> p c g", g=NG),
                axis=AX.X, op=ALU.add)
            j80 = T("j80", 80)
            nc.vector.tensor_tensor(out=j80[:], in0=red80[:], in1=wq80,
                                    op=ALU.mult)
            nc.vector.tensor_reduce(
                out=partials[:, COL_CLS:COL_CLS + 1], in_=j80[:], axis=AX.X,
                op=ALU.add)
            # corr: -(h1*d + P) * selw summed
            f1n = T("f1n", 12, bf16)
            nc.vector.tensor_tensor(out=f1n[:], in0=h1[:],
                                    in1=d_all[:, P_SEL:P_SEL + 12],
                                    op=ALU.mult)
            ncor = T("ncor", 12, bf16)
            nc.vector.tensor_tensor(out=ncor[:], in0=f1n[:],
                                    in1=P_all[:, P_SEL:P_SEL + 12],
                                    op=ALU.add)
            ncm = T("ncm", 12)
            nc.vector.tensor_scalar_mul(ncm[:], ncor[:], -1.0)
            jc = T("jc", 12)
            nc.vector.tensor_tensor(out=jc[:], in0=ncm[:], in1=selw,
                                    op=ALU.mult)
            nc.vector.tensor_reduce(
                out=partials[:, COL_CORR:COL_CORR + 1], in_=jc[:], axis=AX.X,
                op=ALU.add)

            nc.sync.dma_start(out=outp[:], in_=partials[:])

    _split_multi_waits(nc)
    return nc


def _build(mode):
    if mode == "v1nopool":
        return _build_v1(use_pool=False, use_accum=False)
    if mode == "v1min":
        return _build_v1(use_pool=False, use_accum=False)
    if mode == "v1accum":
        return _build_v1(use_accum=True)
    if mode == "v1":
        return _build_v1(use_accum=False)
    if mode == "v2":
        return _build_v2()
    # default: v3
    return _build_v3()


def _host_prepare(p_raw, labels, label_mask, cls_weight):
    """Replicate reference.assign_targets on host; build per-core device
    inputs.  Returns (ch4, posc2, aux, n_targets, n_pos)."""
    labels = np.asarray(labels, dtype=np.float32)
    mask = np.asarray(label_mask).astype(bool)
    cw = np.asarray(cls_weight, dtype=np.float32)

    gcls = labels[..., 0].astype(np.int32)
    gx = labels[..., 1] * IMG
    gy = labels[..., 2] * IMG
    gw = labels[..., 3] * IMG
    gh = labels[..., 4] * IMG
    gi = np.clip(gx / STRIDE, np.float32(0.0),
                 np.float32(W - 0.001)).astype(np.int32)
    gj = np.clip(gy / STRIDE, np.float32(0.0),
                 np.float32(H - 0.001)).astype(np.int32)
    gtw, gth = gw / STRIDE, gh / STRIDE
    ag = ANCHORS / STRIDE
    inter = (np.minimum(gtw[..., None], ag[:, 0])
             * np.minimum(gth[..., None], ag[:, 1]))
    union = (gtw[..., None] * gth[..., None] + ag[:, 0] * ag[:, 1]
             - inter + np.float32(1e-9))
    best_a = np.argmax(inter / union, axis=-1).astype(np.int32)

    offs = [(di, dj) for di in (-1, 0, 1) for dj in (-1, 0, 1)]
    # ordered scatter: tbox last-write-wins, tcls accumulates the class set
    targets = {}  # (b, a, j, i) -> [set(cls), (bx, by, bw, bh)]
    for b in range(B):
        for m in range(M):
            if not mask[b, m]:
                continue
            a = int(best_a[b, m])
            c = int(gcls[b, m])
            box = (gx[b, m], gy[b, m], gw[b, m], gh[b, m])
            for di, dj in offs:
                i = min(max(int(gi[b, m]) + di, 0), W - 1)
                j = min(max(int(gj[b, m]) + dj, 0), H - 1)
                e = targets.setdefault((b, a, j, i), [set(), None])
                e[0].add(c)
                e[1] = box
    n_targets = len(targets)
    n_pos = max(n_targets, 1)

    ch4 = np.ascontiguousarray(
        np.asarray(p_raw, dtype=np.float32)[..., 4]
    ).reshape(NCORES, 128, KD)

    pr = np.asarray(p_raw, dtype=np.float32).reshape(NCORES, BL, NA, H, W,
                                                     5 + C)
    posc = np.full((NCORES, 128, C, NG), EMPTY_CLS, dtype=np.float32)
    sel = np.zeros((NCORES, 128, NSEL), dtype=np.float32)
    box4 = np.zeros((NCORES, 128, 4, NG), dtype=np.float32)
    aux = np.zeros((NCORES, 128, AUXW), dtype=np.float32)
    aux[:, :, A_AWH:A_AWH + 12] = 1.0        # empty slots: pw=ph=1 (no /0)
    aux[:, :, A_AREA:A_AREA + 6] = float(EPS)
    aux[:, :, A_WQ:A_WQ + 80] = cw

    w_obj = 0.25 / float(NTOT)
    w_cls = 0.125 / (float(n_pos) * C)

    slot_ctr = [0] * NCORES
    sel_ctr = [0] * NCORES
    for (b, a, j, i), (clsset, box) in targets.items():
        core = b // BL
        s = slot_ctr[core]
        slot_ctr[core] += 1
        assert s < 128 * NG, "positive-slot capacity exceeded"
        p_, g_ = s % 128, s // 128
        bloc = b - core * BL
        row = pr[core, bloc, a, j, i]
        box4[core, p_, :, g_] = row[0:4]
        posc[core, p_, :, g_] = row[5:]
        bx, by, bw, bh = box
        gx1 = bx - bw * np.float32(0.5)
        gx2 = bx + bw * np.float32(0.5)
        gy1 = by - bh * np.float32(0.5)
        gy2 = by + bh * np.float32(0.5)
        areag = (max(gx2 - gx1, np.float32(0.0))
                 * max(gy2 - gy1, np.float32(0.0)))
        au = aux[core, p_]
        au[A_CXY + g_] = 8.0 * i + 8.0
        au[A_CXY + 6 + g_] = 8.0 * j + 8.0
        au[A_AWH + g_] = ANCHORS[a, 0]
        au[A_AWH + 6 + g_] = ANCHORS[a, 1]
        au[A_G1 + g_] = gx1
        au[A_G1 + 6 + g_] = gy1
        au[A_G2 + g_] = gx2
        au[A_G2 + 6 + g_] = gy2
        au[A_GM + g_] = bx
        au[A_GM + 6 + g_] = by
        au[A_AREA + g_] = areag + EPS
        au[A_ATG + g_] = np.arctan(bw / (bh + EPS))
        au[A_VALID + g_] = 1.0
        # correction entries: objectness (t=1) + each target class (t=1)
        t = sel_ctr[core]
        sel_ctr[core] += 1 + len(clsset)
        assert sel_ctr[core] <= 128 * NSEL, "correction capacity exceeded"
        sel[core, t % 128, t // 128] = row[4]
        aux[core, t % 128, A_SELW + t // 128] = w_obj
        for c in clsset:
            t += 1
            sel[core, t % 128, t // 128] = row[5 + c]
            aux[core, t % 128, A_SELW + t // 128] = w_cls * cw[c]

    posc2 = np.concatenate(
        [posc.reshape(NCORES, 128, C * NG), sel,
         box4.reshape(NCORES, 128, 4 * NG)], axis=2)
    return ch4, np.ascontiguousarray(posc2), aux, n_targets, n_pos




def _host_prepare_v3(p_raw, labels, label_mask, cls_weight):
    import ml_dtypes
    ch4, posc2, aux, n_targets, n_pos = _host_prepare(
        p_raw, labels, label_mask, cls_weight)
    aux3 = np.zeros((NCORES, 128, AUX3), dtype=np.float32)
    aux3[:, :, B_POS4:B_POS4 + 24] = posc2[:, :, P_BOX:PCW]
    aux3[:, :, B_CXY:B_CXY + 12] = aux[:, :, A_CXY:A_CXY + 12]
    aux3[:, :, B_AWH:B_AWH + 12] = aux[:, :, A_AWH:A_AWH + 12]
    aux3[:, :, B_G1:B_G1 + 12] = aux[:, :, A_G1:A_G1 + 12]
    aux3[:, :, B_G2:B_G2 + 12] = aux[:, :, A_G2:A_G2 + 12]
    aux3[:, :, B_GM:B_GM + 12] = aux[:, :, A_GM:A_GM + 12]
    aux3[:, :, B_AREA:B_AREA + 6] = aux[:, :, A_AREA:A_AREA + 6]
    aux3[:, :, B_VALID:B_VALID + 6] = aux[:, :, A_VALID:A_VALID + 6]
    aux3[:, :, B_SELW:B_SELW + 12] = aux[:, :, A_SELW:A_SELW + 12]
    aux3[:, :, B_WQ:B_WQ + 80] = aux[:, :, A_WQ:A_WQ + 80]
    # resolve the atan range-fix branch on host: the sign of
    # (atan(q) - atan(gw/gh)) flips under q -> 1/q reflection but the
    # square is invariant, so upload atg or pi/2-atg per slot
    x2 = posc2[:, :, P_BOX + 12:P_BOX + 18].astype(np.float64)
    x3 = posc2[:, :, P_BOX + 18:P_BOX + 24].astype(np.float64)
    aw = aux[:, :, A_AWH:A_AWH + 6].astype(np.float64)
    ah = aux[:, :, A_AWH + 6:A_AWH + 12].astype(np.float64)
    w = x2 + np.log(aw) - x3 - np.log(ah)
    atg = aux[:, :, A_ATG:A_ATG + 6].astype(np.float64)
    aux3[:, :, B_ATGX:B_ATGX + 6] = np.where(
        w > 0, np.pi / 2 - atg, atg).astype(np.float32)
    big = np.concatenate([posc2[:, :, 0:P_SEL + 12], ch4], axis=2)
    big = np.ascontiguousarray(big.astype(ml_dtypes.bfloat16))
    return aux3, big, n_targets, n_pos


def kernel(p_raw, labels, label_mask, cls_weight):
    global LAST_RESULT
    if MODE.startswith("v3"):
        aux3, big, n_targets, n_pos = _host_prepare_v3(
            p_raw, labels, label_mask, cls_weight)
        in_maps = [{"aux": aux3[c], "big": big[c]} for c in range(NCORES)]
    else:
        ch4, posc2, aux, n_targets, n_pos = _host_prepare(
            p_raw, labels, label_mask, cls_weight)
        in_maps = [
            {"ch4": ch4[c], "posc2": posc2[c], "aux": aux[c]}
            for c in range(NCORES)
        ]

    if MODE not in _BUILD_CACHE:
        _BUILD_CACHE[MODE] = _build(MODE)
    nc = _BUILD_CACHE[MODE]
    r = run_bass_kernel_spmd(
        nc, in_maps, core_ids=list(range(NCORES)), trace=TRACE, **TRACE_KW
    )
    LAST_RESULT = r

    outs = np.stack([np.asarray(r.results[c]["out"]) for c in range(NCORES)])
    s = outs.astype(np.float64).sum(axis=(0, 1))
    total = (7.5 * (n_targets + s[COL_BOX]) / n_pos
             + 0.25 / NTOT * s[COL_OBJ]
             + 0.125 / (n_pos * C) * s[COL_CLS]
             + s[COL_CORR])
    return np.float32(total)
